# revision 27
# baseline (speedup 1.0000x reference)
"""GRU cell (EncoderRNN single step) on 8 Trainium2 NeuronCores.

Full inputs -> full output. Sharding: each core owns a 256-wide slice of the
hidden dimension across all three gates (rows of w_ih/w_hh); no collectives.
The host gathers the embedding row (only that row of the table is needed) and
concatenates the 8 per-core h_new slices.

All matrix-vector work runs on the PE array via host-transposed k-major fp16
weights (measured LDWEIGHTS/MATMUL pair for [128,128]x[128,1]: ~27ns warm, so
the 384 pairs/core hide under the weight stream). DVE only does ten [128,2]
gate ops; ACT does the three sigmoids (tanh(v) = 2*sigmoid(2v)-1 keeps a
single act-table set).

Streaming: 6.29MB/core of fp16 weights over THREE DMA rows - the 16 SDMA
engines round-robin between rows at packet granularity, and a single row
leaves them ~50% idle on descriptor-packet fetch, so more rows = more
overlap. r rides first on the two HWDGE rings, z last on them, n on the
Pool SWDGE row (which starts earlier and lands mid-stream), so each gate's
sigmoid chain overlaps the next gate's stream.

Hard-won correctness rules (cold-run races otherwise, masked on reruns by
stale-but-identical SBUF/PSUM contents):
- ONE semaphore per DMA. A shared sem with >=16*(c+1) thresholds is unsound:
  the 16 SDMA engines' receipt increments all land in one counter, so a
  straggler engine can still be writing chunk c while faster engines push
  the sum past the threshold.
- The gate-complete sem must not fire before the last matmul's ~128-cycle
  PSUM drain lands: a dummy 192-column matmul (fence) carries the inc.
- start=True clears the has_written state for the whole 2KB PSUM bank
  region: only the FIRST matmul of each accumulator bank sets it.
"""

import sys

if "/opt/trn_rl_repo" not in sys.path:
    sys.path.insert(0, "/opt/trn_rl_repo")

import numpy as np
import ml_dtypes

H = 2048
NCORES = 8
HC = H // NCORES          # 256 hidden elems per core
UT = HC // 128            # 2 columns for the per-core [128, 2] gate slices
KC = H // 128             # 16 k-chunks
GMW = KC * HC             # 4096 cols per gate-matrix image ([128, 4096] f16)

_CACHE = {}


def _build():
    import contextlib
    from concourse import bacc, bass, mybir

    class _BareBlock(bass.BassBlock):
        # Skip the exit drains + all-engine EVSEM barrier: every cross-engine
        # dependency is semaphore-guarded and the issuing engine of the hout
        # DMA waits for its receipt, so nothing needs a terminal rendezvous.
        def __exit__(self, exc_type, exc_val, exc_tb):
            if exc_type is None:
                for engine, last_body in self.last_body.items():
                    with self.bass.body(
                        last_body, parent=self.bass.cur_bb, allow_existing_parent=True
                    ):
                        engine.br(self.end_bb)
                self.bass.switch_bb(self.end_bb)

    @contextlib.contextmanager
    def bare_block(nc):
        assert nc.cur_block is None
        with _BareBlock(nc, f"block_{nc.next_id()}") as blk:
            nc.cur_block = blk
            yield blk
        nc.cur_block = None

    f32 = mybir.dt.float32
    f16 = mybir.dt.float16
    Alu = mybir.AluOpType
    Act = mybir.ActivationFunctionType

    nc = bacc.Bacc(
        "TRN2",
        target_bir_lowering=False,
        debug=False,
        num_devices=NCORES,
        detect_race_conditions=False,
    )

    # ring images (fp16, 4KB rows): sync [r_wih | z_wih], scalar [r_whh |
    # z_whh], pool [n_wih | n_whh]; each gate-matrix slab [128, 4096]
    wa_d = nc.dram_tensor("wa", [128, 2 * GMW], f16, kind="ExternalInput")
    wb_d = nc.dram_tensor("wb", [128, 2 * GMW], f16, kind="ExternalInput")
    wn_d = nc.dram_tensor("wn", [128, 2 * GMW], f16, kind="ExternalInput")
    xc_d = nc.dram_tensor("xc", [128, KC], f16, kind="ExternalInput")
    hc_d = nc.dram_tensor("hc", [128, KC], f16, kind="ExternalInput")
    # cols: brz[0:2*UT], bin[2*UT:3*UT], bhn[3*UT:4*UT], hs[4*UT:5*UT]
    smalls = nc.dram_tensor("smalls", [128, 5 * UT], f32, kind="ExternalInput")
    hout = nc.dram_tensor("hout", [128, UT], f32, kind="ExternalOutput")

    sb = lambda name, shape, dt=f32: nc.alloc_sbuf_tensor(name, list(shape), dt).ap()
    wRi = sb("wRi", [128, GMW], f16)
    wRh = sb("wRh", [128, GMW], f16)
    wNi = sb("wNi", [128, GMW], f16)
    wNh = sb("wNh", [128, GMW], f16)
    wZi = sb("wZi", [128, GMW], f16)
    wZh = sb("wZh", [128, GMW], f16)
    xc = sb("xc_s", [128, KC], f16)
    hc = sb("hc_s", [128, KC], f16)
    sm = sb("sm", [128, 5 * UT])
    brz_t = sm[:, 0 : 2 * UT]
    bin_t = sm[:, 2 * UT : 3 * UT]
    bhn_t = sm[:, 3 * UT : 4 * UT]
    hs_t = sm[:, 4 * UT : 5 * UT]
    rp = sb("rp", [128, UT])
    zp = sb("zp", [128, UT])
    r_t = sb("r_t", [128, UT])
    z_t = sb("z_t", [128, UT])
    hnb = sb("hnb", [128, UT])
    t3 = sb("t3", [128, UT])
    t4 = sb("t4", [128, UT])
    s_tile = sb("s_tile", [128, UT])   # sigmoid(2v) for the n gate
    n_t = sb("n_t", [128, UT])         # u = 1 - 2s = -n
    t5 = sb("t5", [128, UT])
    hnew = sb("hnew", [128, UT])

    pe_fence = nc.alloc_psum_tensor("pe_fence", [128, 192], f32).ap()
    grp = nc.alloc_psum_tensor("grp", [128, UT], f32).ap()    # gi_r + gh_r
    gzp = nc.alloc_psum_tensor("gzp", [128, UT], f32).ap()    # gi_z + gh_z
    gin_p = nc.alloc_psum_tensor("gin_p", [128, UT], f32).ap()
    ghn_p = nc.alloc_psum_tensor("ghn_p", [128, UT], f32).ap()

    with contextlib.ExitStack() as _stack:
        sem = lambda n: _stack.enter_context(nc.semaphore(n))
        s_x = sem("s_x")
        s_h = sem("s_h")
        s_sm = sem("s_sm")
        s_wri = sem("s_wri")
        s_wrh = sem("s_wrh")
        s_wni = sem("s_wni")
        s_wnh = sem("s_wnh")
        s_wzi = sem("s_wzi")
        s_wzh = sem("s_wzh")
        s_gr = sem("s_gr")
        s_gn = sem("s_gn")
        s_gz = sem("s_gz")
        s_dve = sem("s_dve")
        s_act = sem("s_act")
        s_out = sem("s_out")
        block = _stack.enter_context(bare_block(nc))

        @block.sync
        def _(sync):
            sync.dma_start(out=xc[:, :], in_=xc_d.ap()[:, :]).then_inc(s_x, 16)
            sync.dma_start(out=sm[:, :], in_=smalls.ap()[:, :]).then_inc(s_sm, 16)
            sync.dma_start(out=wRi[:, :], in_=wa_d.ap()[:, 0:GMW]).then_inc(s_wri, 16)
            sync.dma_start(out=wZi[:, :], in_=wa_d.ap()[:, GMW : 2 * GMW]).then_inc(
                s_wzi, 16
            )

        @block.scalar
        def _(scalar):
            scalar.dma_start(out=hc[:, :], in_=hc_d.ap()[:, :]).then_inc(s_h, 16)
            scalar.dma_start(out=wRh[:, :], in_=wb_d.ap()[:, 0:GMW]).then_inc(
                s_wrh, 16
            )
            scalar.dma_start(out=wZh[:, :], in_=wb_d.ap()[:, GMW : 2 * GMW]).then_inc(
                s_wzh, 16
            )
            # r-gate sigmoid
            scalar.wait_ge(s_dve, 1)
            nc.scalar.activation(out=r_t[:, :], in_=rp[:, :], func=Act.Sigmoid).then_inc(
                s_act, 1
            )
            # n-gate tanh(v) = 2*sigmoid(2v) - 1, affine folded into DVE ops
            scalar.wait_ge(s_dve, 5)
            nc.scalar.activation(
                out=s_tile[:, :], in_=t4[:, :], func=Act.Sigmoid, scale=2.0
            ).then_inc(s_act, 1)
            # z-gate sigmoid
            scalar.wait_ge(s_dve, 8)
            nc.scalar.activation(out=z_t[:, :], in_=zp[:, :], func=Act.Sigmoid).then_inc(
                s_act, 1
            )
            scalar.wait_ge(s_dve, 10)
            scalar.dma_start(out=hout.ap()[:, :], in_=hnew[:, :]).then_inc(s_out, 16)
            scalar.wait_ge(s_out, 16)

        @block.gpsimd
        def _(gpsimd):
            gpsimd.dma_start(out=wNi[:, :], in_=wn_d.ap()[:, 0:GMW]).then_inc(
                s_wni, 16
            )
            gpsimd.dma_start(out=wNh[:, :], in_=wn_d.ap()[:, GMW : 2 * GMW]).then_inc(
                s_wnh, 16
            )

        @block.tensor
        def _(tensor):
            def fence():
                return nc.tensor.matmul(
                    pe_fence[:, :],
                    lhsT=wRi[:, 0:128],
                    rhs=wRi[:, 0:192],
                    start=True,
                    stop=True,
                )

            def half_pairs(wt, vec, acc, start, stop):
                # 32 pairs: one gate-matrix slab [128, 16 k-tiles * 256]
                last = None
                for t in range(KC):
                    for j in range(UT):
                        last = nc.tensor.matmul(
                            acc[:, j : j + 1],
                            lhsT=wt[:, t * HC + j * 128 : t * HC + (j + 1) * 128],
                            rhs=vec[:, t : t + 1],
                            start=(start and t == 0 and j == 0),
                            stop=(stop and t == KC - 1),
                        )
                return last

            tensor.wait_ge(s_x, 16)
            tensor.wait_ge(s_h, 16)
            tensor.wait_ge(s_wri, 16)
            half_pairs(wRi, xc, grp, start=True, stop=False)
            tensor.wait_ge(s_wrh, 16)
            half_pairs(wRh, hc, grp, start=False, stop=True)
            fence().then_inc(s_gr, 1)
            tensor.wait_ge(s_wni, 16)
            half_pairs(wNi, xc, gin_p, start=True, stop=True)
            tensor.wait_ge(s_wnh, 16)
            half_pairs(wNh, hc, ghn_p, start=True, stop=True)
            fence().then_inc(s_gn, 1)
            tensor.wait_ge(s_wzi, 16)
            half_pairs(wZi, xc, gzp, start=True, stop=False)
            tensor.wait_ge(s_wzh, 16)
            half_pairs(wZh, hc, gzp, start=False, stop=True)
            fence().then_inc(s_gz, 1)

        @block.vector
        def _(vector):
            vector.wait_ge(s_gr, 1)
            vector.wait_ge(s_sm, 16)
            nc.vector.tensor_tensor(
                out=rp[:, :], in0=grp[:, :], in1=brz_t[:, 0:UT], op=Alu.add
            ).then_inc(s_dve, 1)  # 1 -> ACT sigmoid(r)
            vector.wait_ge(s_gn, 1)
            nc.vector.tensor_tensor(
                out=hnb[:, :], in0=ghn_p[:, :], in1=bhn_t[:, :], op=Alu.add
            ).then_inc(s_dve, 1)  # 2
            nc.vector.tensor_tensor(
                out=t4[:, :], in0=gin_p[:, :], in1=bin_t[:, :], op=Alu.add
            ).then_inc(s_dve, 1)  # 3
            vector.wait_ge(s_act, 1)
            vector.wait_ge(s_dve, 3)
            nc.vector.tensor_tensor(
                out=t3[:, :], in0=r_t[:, :], in1=hnb[:, :], op=Alu.mult
            ).then_inc(s_dve, 1)  # 4
            vector.wait_ge(s_dve, 4)
            nc.vector.tensor_tensor(
                out=t4[:, :], in0=t4[:, :], in1=t3[:, :], op=Alu.add
            ).then_inc(s_dve, 1)  # 5 -> ACT sigmoid(2v)
            vector.wait_ge(s_act, 2)
            # u = 1 - 2s = -n
            nc.vector.tensor_scalar(
                out=n_t[:, :], in0=s_tile[:, :], scalar1=-2.0, scalar2=1.0,
                op0=Alu.mult, op1=Alu.add,
            ).then_inc(s_dve, 1)  # 6
            vector.wait_ge(s_dve, 6)
            nc.vector.tensor_tensor(
                out=t5[:, :], in0=hs_t[:, :], in1=n_t[:, :], op=Alu.add
            ).then_inc(s_dve, 1)  # 7  (t5 = hs - n)
            vector.wait_ge(s_gz, 1)
            nc.vector.tensor_tensor(
                out=zp[:, :], in0=gzp[:, :], in1=brz_t[:, UT : 2 * UT], op=Alu.add
            ).then_inc(s_dve, 1)  # 8 -> ACT sigmoid(z)
            vector.wait_ge(s_act, 3)
            vector.wait_ge(s_dve, 7)
            nc.vector.tensor_tensor(
                out=t5[:, :], in0=z_t[:, :], in1=t5[:, :], op=Alu.mult
            ).then_inc(s_dve, 1)  # 9  (t5 = z * (hs - n))
            vector.wait_ge(s_dve, 9)
            nc.vector.tensor_tensor(
                out=hnew[:, :], in0=t5[:, :], in1=n_t[:, :], op=Alu.subtract
            ).then_inc(s_dve, 1)  # 10  (hnew = n + z*(hs - n))

    nc.compile()

    # Post-compile surgery:
    # 1. Strip the entry all-engine barrier (per-engine Drain + barrier_*
    #    EventSemaphores in the entry block). The only ordering it provides
    #    is Pool's preamble memsets vs other engines' const-AP reads; the
    #    first such read (ACT sigmoid bias) is ~15us after the ~1us memsets.
    blk0 = nc.main_func.blocks[0]
    kill = [
        i
        for i in blk0.instructions
        if isinstance(i, mybir.InstDrain)
        or (isinstance(i, mybir.InstEventSemaphore) and "barrier_" in str(i))
    ]
    assert len(kill) >= 10, f"expected entry barrier instrs, got {len(kill)}"
    for i in kill:
        blk0.instructions.remove(i)
    # 2. Move the entry LoadActFuncSet after the scalar-ring DMA issues so
    #    its 1.3us table load does not delay the ring start; the set that
    #    the sigmoids use loads right before them anyway.
    for b in nc.main_func.blocks:
        loads = [i for i in b.instructions if isinstance(i, mybir.InstLoadActFuncSet)]
        if len(loads) >= 2:
            first = loads[0]
            b.instructions.remove(first)
            dmas = [
                idx
                for idx, i in enumerate(b.instructions)
                if isinstance(i, mybir.InstDMACopy)
            ]
            b.instructions.insert(dmas[-1] + 1, first)
    return nc


def get_nc():
    if "nc" not in _CACHE:
        _CACHE["nc"] = _build()
    return _CACHE["nc"]


def make_in_maps(inputs):
    """Host-side sharding: full-input dict -> 8 per-core input maps."""
    emb = np.asarray(inputs["emb"], dtype=np.float32)
    w_ih = np.asarray(inputs["w_ih"], dtype=np.float32)
    w_hh = np.asarray(inputs["w_hh"], dtype=np.float32)
    b_ih = np.asarray(inputs["b_ih"], dtype=np.float32)
    b_hh = np.asarray(inputs["b_hh"], dtype=np.float32)
    idx = int(np.asarray(inputs["input"]).reshape(-1)[0])
    x = np.ascontiguousarray(emb[idx])
    h = np.asarray(inputs["hidden"], dtype=np.float32).reshape(H)

    xc_host = np.ascontiguousarray(x.reshape(KC, 128).T.astype(np.float16))
    hc_host = np.ascontiguousarray(h.reshape(KC, 128).T.astype(np.float16))
    bsum = b_ih + b_hh

    in_maps = []
    for c in range(NCORES):
        # per-core row slices, PyTorch gate order r, z, n
        sl = [slice(g * H + c * HC, g * H + c * HC + HC) for g in range(3)]
        r_sl, z_sl, n_sl = sl[0], sl[1], sl[2]

        # k-major gate-matrix image [128, 16*256]:
        # img[p, t*256 + o] = G^T[t*128 + p, o]
        def img(m, g_sl):
            rows = m[g_sl].T.astype(np.float16)          # [2048, 256]
            return rows.reshape(KC, 128, HC).transpose(1, 0, 2).reshape(128, GMW)

        wa_c = np.ascontiguousarray(
            np.concatenate([img(w_ih, r_sl), img(w_ih, z_sl)], axis=1)
        )
        wb_c = np.ascontiguousarray(
            np.concatenate([img(w_hh, r_sl), img(w_hh, z_sl)], axis=1)
        )
        wn_c = np.ascontiguousarray(
            np.concatenate([img(w_ih, n_sl), img(w_hh, n_sl)], axis=1)
        )
        brz_c = np.concatenate([bsum[r_sl], bsum[z_sl]]).reshape(2 * UT, 128).T
        bin_c = b_ih[n_sl].reshape(UT, 128).T
        bhn_c = b_hh[n_sl].reshape(UT, 128).T
        hs_c = h[c * HC : (c + 1) * HC].reshape(UT, 128).T
        smalls_c = np.ascontiguousarray(
            np.concatenate([brz_c, bin_c, bhn_c, hs_c], axis=1), dtype=np.float32
        )
        in_maps.append(
            {
                "wa": wa_c,
                "wb": wb_c,
                "wn": wn_c,
                "xc": xc_host,
                "hc": hc_host,
                "smalls": smalls_c,
            }
        )
    return in_maps


def run_on_hw(in_maps, trace=False):
    from concourse.bass_utils import run_bass_kernel_spmd

    kwargs = {}
    if trace:
        kwargs.update(trace=True, trace_cores=list(range(NCORES)))
    return run_bass_kernel_spmd(get_nc(), in_maps, core_ids=list(range(NCORES)), **kwargs)


def assemble(results):
    h_new = np.concatenate(
        [np.ascontiguousarray(results[c]["hout"].T).reshape(HC) for c in range(NCORES)]
    )
    out = h_new.reshape(1, 1, H).astype(np.float32)
    return out, out.copy()


def kernel(**inputs):
    in_maps = make_in_maps(inputs)
    res = run_on_hw(in_maps)
    return assemble(res.results)


# revision 28
# speedup vs baseline: 1.1065x; 1.1065x over previous
"""GRU cell (EncoderRNN single step) on 8 Trainium2 NeuronCores.

Full inputs -> full output. Sharding: each core owns a 256-wide slice of the
hidden dimension across all three gates (rows of w_ih/w_hh); no collectives.
The host gathers the embedding row (only that row of the table is needed) and
concatenates the 8 per-core h_new slices.

All matrix-vector work runs on the PE array via host-transposed k-major fp16
weights (measured LDWEIGHTS/MATMUL pair for [128,128]x[128,1]: ~27ns warm, so
the 384 pairs/core hide under the weight stream). DVE only does ten [128,2]
gate ops; ACT does the three sigmoids (tanh(v) = 2*sigmoid(2v)-1 keeps a
single act-table set).

Streaming: 6.29MB/core of fp16 weights over THREE DMA rows - the 16 SDMA
engines round-robin between rows at packet granularity, and a single row
leaves them ~50% idle on descriptor-packet fetch, so more rows = more
overlap. r rides first on the two HWDGE rings, z last on them, n on the
Pool SWDGE row (which starts earlier and lands mid-stream), so each gate's
sigmoid chain overlaps the next gate's stream.

Hard-won correctness rules (cold-run races otherwise, masked on reruns by
stale-but-identical SBUF/PSUM contents):
- ONE semaphore per DMA. A shared sem with >=16*(c+1) thresholds is unsound:
  the 16 SDMA engines' receipt increments all land in one counter, so a
  straggler engine can still be writing chunk c while faster engines push
  the sum past the threshold.
- The gate-complete sem must not fire before the last matmul's ~128-cycle
  PSUM drain lands: a dummy 192-column matmul (fence) carries the inc.
- start=True clears the has_written state for the whole 2KB PSUM bank
  region: only the FIRST matmul of each accumulator bank sets it.
"""

import sys

if "/opt/trn_rl_repo" not in sys.path:
    sys.path.insert(0, "/opt/trn_rl_repo")

import numpy as np
import ml_dtypes

H = 2048
NCORES = 8
HC = H // NCORES          # 256 hidden elems per core
UT = HC // 128            # 2 columns for the per-core [128, 2] gate slices
KC = H // 128             # 16 k-chunks
GMW = KC * HC             # 4096 cols per gate-matrix image ([128, 4096] f16)

_CACHE = {}


def _build():
    import contextlib
    from concourse import bacc, bass, mybir

    class _BareBlock(bass.BassBlock):
        # Skip the exit drains + all-engine EVSEM barrier: every cross-engine
        # dependency is semaphore-guarded and the issuing engine of the hout
        # DMA waits for its receipt, so nothing needs a terminal rendezvous.
        def __exit__(self, exc_type, exc_val, exc_tb):
            if exc_type is None:
                for engine, last_body in self.last_body.items():
                    with self.bass.body(
                        last_body, parent=self.bass.cur_bb, allow_existing_parent=True
                    ):
                        engine.br(self.end_bb)
                self.bass.switch_bb(self.end_bb)

    @contextlib.contextmanager
    def bare_block(nc):
        assert nc.cur_block is None
        with _BareBlock(nc, f"block_{nc.next_id()}") as blk:
            nc.cur_block = blk
            yield blk
        nc.cur_block = None

    f32 = mybir.dt.float32
    f16 = mybir.dt.float16
    Alu = mybir.AluOpType
    Act = mybir.ActivationFunctionType

    nc = bacc.Bacc(
        "TRN2",
        target_bir_lowering=False,
        debug=False,
        num_devices=NCORES,
        detect_race_conditions=False,
    )

    # one CONTIGUOUS 1MB DRAM tensor per gate-matrix slab: concatenating
    # slabs into a wider image makes every descriptor read 8KB then skip
    # 8KB, halving DRAM page locality (measured 16GB/s vs 25GB/s per queue)
    wri_d = nc.dram_tensor("wri", [128, GMW], f16, kind="ExternalInput")
    wrh_d = nc.dram_tensor("wrh", [128, GMW], f16, kind="ExternalInput")
    wni_d = nc.dram_tensor("wni", [128, GMW], f16, kind="ExternalInput")
    wnh_d = nc.dram_tensor("wnh", [128, GMW], f16, kind="ExternalInput")
    wzi_d = nc.dram_tensor("wzi", [128, GMW], f16, kind="ExternalInput")
    wzh_d = nc.dram_tensor("wzh", [128, GMW], f16, kind="ExternalInput")
    xc_d = nc.dram_tensor("xc", [128, KC], f16, kind="ExternalInput")
    hc_d = nc.dram_tensor("hc", [128, KC], f16, kind="ExternalInput")
    # cols: brz[0:2*UT], bin[2*UT:3*UT], bhn[3*UT:4*UT], hs[4*UT:5*UT]
    smalls = nc.dram_tensor("smalls", [128, 5 * UT], f32, kind="ExternalInput")
    hout = nc.dram_tensor("hout", [128, UT], f32, kind="ExternalOutput")

    sb = lambda name, shape, dt=f32: nc.alloc_sbuf_tensor(name, list(shape), dt).ap()
    wRi = sb("wRi", [128, GMW], f16)
    wRh = sb("wRh", [128, GMW], f16)
    wNi = sb("wNi", [128, GMW], f16)
    wNh = sb("wNh", [128, GMW], f16)
    wZi = sb("wZi", [128, GMW], f16)
    wZh = sb("wZh", [128, GMW], f16)
    xc = sb("xc_s", [128, KC], f16)
    hc = sb("hc_s", [128, KC], f16)
    sm = sb("sm", [128, 5 * UT])
    brz_t = sm[:, 0 : 2 * UT]
    bin_t = sm[:, 2 * UT : 3 * UT]
    bhn_t = sm[:, 3 * UT : 4 * UT]
    hs_t = sm[:, 4 * UT : 5 * UT]
    rp = sb("rp", [128, UT])
    zp = sb("zp", [128, UT])
    r_t = sb("r_t", [128, UT])
    z_t = sb("z_t", [128, UT])
    hnb = sb("hnb", [128, UT])
    t3 = sb("t3", [128, UT])
    t4 = sb("t4", [128, UT])
    s_tile = sb("s_tile", [128, UT])   # sigmoid(2v) for the n gate
    n_t = sb("n_t", [128, UT])         # u = 1 - 2s = -n
    t5 = sb("t5", [128, UT])
    hnew = sb("hnew", [128, UT])

    pe_fence = nc.alloc_psum_tensor("pe_fence", [128, 192], f32).ap()
    grp = nc.alloc_psum_tensor("grp", [128, UT], f32).ap()    # gi_r + gh_r
    gzp = nc.alloc_psum_tensor("gzp", [128, UT], f32).ap()    # gi_z + gh_z
    gin_p = nc.alloc_psum_tensor("gin_p", [128, UT], f32).ap()
    ghn_p = nc.alloc_psum_tensor("ghn_p", [128, UT], f32).ap()

    with contextlib.ExitStack() as _stack:
        sem = lambda n: _stack.enter_context(nc.semaphore(n))
        s_x = sem("s_x")
        s_h = sem("s_h")
        s_sm = sem("s_sm")
        s_wri = sem("s_wri")
        s_wrh = sem("s_wrh")
        s_wni = sem("s_wni")
        s_wnh = sem("s_wnh")
        s_wzi = sem("s_wzi")
        s_wzh = sem("s_wzh")
        s_gr = sem("s_gr")
        s_gn = sem("s_gn")
        s_gz = sem("s_gz")
        s_dve = sem("s_dve")
        s_act = sem("s_act")
        s_out = sem("s_out")
        block = _stack.enter_context(bare_block(nc))

        @block.sync
        def _(sync):
            sync.dma_start(out=xc[:, :], in_=xc_d.ap()[:, :]).then_inc(s_x, 16)
            sync.dma_start(out=sm[:, :], in_=smalls.ap()[:, :]).then_inc(s_sm, 16)
            sync.dma_start(out=wRi[:, :], in_=wri_d.ap()[:, :]).then_inc(s_wri, 16)
            sync.dma_start(out=wZi[:, :], in_=wzi_d.ap()[:, :]).then_inc(s_wzi, 16)

        @block.scalar
        def _(scalar):
            scalar.dma_start(out=hc[:, :], in_=hc_d.ap()[:, :]).then_inc(s_h, 16)
            scalar.dma_start(out=wRh[:, :], in_=wrh_d.ap()[:, :]).then_inc(s_wrh, 16)
            scalar.dma_start(out=wZh[:, :], in_=wzh_d.ap()[:, :]).then_inc(s_wzh, 16)
            # r-gate sigmoid
            scalar.wait_ge(s_dve, 1)
            nc.scalar.activation(out=r_t[:, :], in_=rp[:, :], func=Act.Sigmoid).then_inc(
                s_act, 1
            )
            # n-gate tanh(v) = 2*sigmoid(2v) - 1, affine folded into DVE ops
            scalar.wait_ge(s_dve, 5)
            nc.scalar.activation(
                out=s_tile[:, :], in_=t4[:, :], func=Act.Sigmoid, scale=2.0
            ).then_inc(s_act, 1)
            # z-gate sigmoid
            scalar.wait_ge(s_dve, 8)
            nc.scalar.activation(out=z_t[:, :], in_=zp[:, :], func=Act.Sigmoid).then_inc(
                s_act, 1
            )
            scalar.wait_ge(s_dve, 10)
            scalar.dma_start(out=hout.ap()[:, :], in_=hnew[:, :]).then_inc(s_out, 16)
            scalar.wait_ge(s_out, 16)

        @block.gpsimd
        def _(gpsimd):
            gpsimd.dma_start(out=wNi[:, :], in_=wni_d.ap()[:, :]).then_inc(s_wni, 16)
            gpsimd.dma_start(out=wNh[:, :], in_=wnh_d.ap()[:, :]).then_inc(s_wnh, 16)

        @block.tensor
        def _(tensor):
            def fence():
                return nc.tensor.matmul(
                    pe_fence[:, :],
                    lhsT=wRi[:, 0:128],
                    rhs=wRi[:, 0:192],
                    start=True,
                    stop=True,
                )

            def half_pairs(wt, vec, acc, start, stop):
                # 32 pairs: one gate-matrix slab [128, 16 k-tiles * 256]
                last = None
                for t in range(KC):
                    for j in range(UT):
                        last = nc.tensor.matmul(
                            acc[:, j : j + 1],
                            lhsT=wt[:, t * HC + j * 128 : t * HC + (j + 1) * 128],
                            rhs=vec[:, t : t + 1],
                            start=(start and t == 0 and j == 0),
                            stop=(stop and t == KC - 1),
                        )
                return last

            tensor.wait_ge(s_x, 16)
            tensor.wait_ge(s_h, 16)
            tensor.wait_ge(s_wri, 16)
            half_pairs(wRi, xc, grp, start=True, stop=False)
            tensor.wait_ge(s_wrh, 16)
            half_pairs(wRh, hc, grp, start=False, stop=True)
            fence().then_inc(s_gr, 1)
            tensor.wait_ge(s_wni, 16)
            half_pairs(wNi, xc, gin_p, start=True, stop=True)
            tensor.wait_ge(s_wnh, 16)
            half_pairs(wNh, hc, ghn_p, start=True, stop=True)
            fence().then_inc(s_gn, 1)
            tensor.wait_ge(s_wzi, 16)
            half_pairs(wZi, xc, gzp, start=True, stop=False)
            tensor.wait_ge(s_wzh, 16)
            half_pairs(wZh, hc, gzp, start=False, stop=True)
            fence().then_inc(s_gz, 1)

        @block.vector
        def _(vector):
            vector.wait_ge(s_gr, 1)
            vector.wait_ge(s_sm, 16)
            nc.vector.tensor_tensor(
                out=rp[:, :], in0=grp[:, :], in1=brz_t[:, 0:UT], op=Alu.add
            ).then_inc(s_dve, 1)  # 1 -> ACT sigmoid(r)
            vector.wait_ge(s_gn, 1)
            nc.vector.tensor_tensor(
                out=hnb[:, :], in0=ghn_p[:, :], in1=bhn_t[:, :], op=Alu.add
            ).then_inc(s_dve, 1)  # 2
            nc.vector.tensor_tensor(
                out=t4[:, :], in0=gin_p[:, :], in1=bin_t[:, :], op=Alu.add
            ).then_inc(s_dve, 1)  # 3
            vector.wait_ge(s_act, 1)
            vector.wait_ge(s_dve, 3)
            nc.vector.tensor_tensor(
                out=t3[:, :], in0=r_t[:, :], in1=hnb[:, :], op=Alu.mult
            ).then_inc(s_dve, 1)  # 4
            vector.wait_ge(s_dve, 4)
            nc.vector.tensor_tensor(
                out=t4[:, :], in0=t4[:, :], in1=t3[:, :], op=Alu.add
            ).then_inc(s_dve, 1)  # 5 -> ACT sigmoid(2v)
            vector.wait_ge(s_act, 2)
            # u = 1 - 2s = -n
            nc.vector.tensor_scalar(
                out=n_t[:, :], in0=s_tile[:, :], scalar1=-2.0, scalar2=1.0,
                op0=Alu.mult, op1=Alu.add,
            ).then_inc(s_dve, 1)  # 6
            vector.wait_ge(s_dve, 6)
            nc.vector.tensor_tensor(
                out=t5[:, :], in0=hs_t[:, :], in1=n_t[:, :], op=Alu.add
            ).then_inc(s_dve, 1)  # 7  (t5 = hs - n)
            vector.wait_ge(s_gz, 1)
            nc.vector.tensor_tensor(
                out=zp[:, :], in0=gzp[:, :], in1=brz_t[:, UT : 2 * UT], op=Alu.add
            ).then_inc(s_dve, 1)  # 8 -> ACT sigmoid(z)
            vector.wait_ge(s_act, 3)
            vector.wait_ge(s_dve, 7)
            nc.vector.tensor_tensor(
                out=t5[:, :], in0=z_t[:, :], in1=t5[:, :], op=Alu.mult
            ).then_inc(s_dve, 1)  # 9  (t5 = z * (hs - n))
            vector.wait_ge(s_dve, 9)
            nc.vector.tensor_tensor(
                out=hnew[:, :], in0=t5[:, :], in1=n_t[:, :], op=Alu.subtract
            ).then_inc(s_dve, 1)  # 10  (hnew = n + z*(hs - n))

    nc.compile()

    # Post-compile surgery:
    # 1. Strip the entry all-engine barrier (per-engine Drain + barrier_*
    #    EventSemaphores in the entry block). The only ordering it provides
    #    is Pool's preamble memsets vs other engines' const-AP reads; the
    #    first such read (ACT sigmoid bias) is ~15us after the ~1us memsets.
    blk0 = nc.main_func.blocks[0]
    kill = [
        i
        for i in blk0.instructions
        if isinstance(i, mybir.InstDrain)
        or (isinstance(i, mybir.InstEventSemaphore) and "barrier_" in str(i))
    ]
    assert len(kill) >= 10, f"expected entry barrier instrs, got {len(kill)}"
    for i in kill:
        blk0.instructions.remove(i)
    # 2. Move the entry LoadActFuncSet after the scalar-ring DMA issues so
    #    its 1.3us table load does not delay the ring start; the set that
    #    the sigmoids use loads right before them anyway.
    for b in nc.main_func.blocks:
        loads = [i for i in b.instructions if isinstance(i, mybir.InstLoadActFuncSet)]
        if len(loads) >= 2:
            first = loads[0]
            b.instructions.remove(first)
            dmas = [
                idx
                for idx, i in enumerate(b.instructions)
                if isinstance(i, mybir.InstDMACopy)
            ]
            b.instructions.insert(dmas[-1] + 1, first)
    return nc


def get_nc():
    if "nc" not in _CACHE:
        _CACHE["nc"] = _build()
    return _CACHE["nc"]


def make_in_maps(inputs):
    """Host-side sharding: full-input dict -> 8 per-core input maps."""
    emb = np.asarray(inputs["emb"], dtype=np.float32)
    w_ih = np.asarray(inputs["w_ih"], dtype=np.float32)
    w_hh = np.asarray(inputs["w_hh"], dtype=np.float32)
    b_ih = np.asarray(inputs["b_ih"], dtype=np.float32)
    b_hh = np.asarray(inputs["b_hh"], dtype=np.float32)
    idx = int(np.asarray(inputs["input"]).reshape(-1)[0])
    x = np.ascontiguousarray(emb[idx])
    h = np.asarray(inputs["hidden"], dtype=np.float32).reshape(H)

    xc_host = np.ascontiguousarray(x.reshape(KC, 128).T.astype(np.float16))
    hc_host = np.ascontiguousarray(h.reshape(KC, 128).T.astype(np.float16))
    bsum = b_ih + b_hh

    in_maps = []
    for c in range(NCORES):
        # per-core row slices, PyTorch gate order r, z, n
        sl = [slice(g * H + c * HC, g * H + c * HC + HC) for g in range(3)]
        r_sl, z_sl, n_sl = sl[0], sl[1], sl[2]

        # k-major gate-matrix image [128, 16*256]:
        # img[p, t*256 + o] = G^T[t*128 + p, o]
        def img(m, g_sl):
            rows = m[g_sl].T.astype(np.float16)          # [2048, 256]
            return rows.reshape(KC, 128, HC).transpose(1, 0, 2).reshape(128, GMW)

        slabs = {
            "wri": img(w_ih, r_sl), "wrh": img(w_hh, r_sl),
            "wni": img(w_ih, n_sl), "wnh": img(w_hh, n_sl),
            "wzi": img(w_ih, z_sl), "wzh": img(w_hh, z_sl),
        }
        slabs = {k: np.ascontiguousarray(v) for k, v in slabs.items()}
        brz_c = np.concatenate([bsum[r_sl], bsum[z_sl]]).reshape(2 * UT, 128).T
        bin_c = b_ih[n_sl].reshape(UT, 128).T
        bhn_c = b_hh[n_sl].reshape(UT, 128).T
        hs_c = h[c * HC : (c + 1) * HC].reshape(UT, 128).T
        smalls_c = np.ascontiguousarray(
            np.concatenate([brz_c, bin_c, bhn_c, hs_c], axis=1), dtype=np.float32
        )
        in_maps.append(
            {
                **slabs,
                "xc": xc_host,
                "hc": hc_host,
                "smalls": smalls_c,
            }
        )
    return in_maps


def run_on_hw(in_maps, trace=False):
    from concourse.bass_utils import run_bass_kernel_spmd

    kwargs = {}
    if trace:
        kwargs.update(trace=True, trace_cores=list(range(NCORES)))
    return run_bass_kernel_spmd(get_nc(), in_maps, core_ids=list(range(NCORES)), **kwargs)


def assemble(results):
    h_new = np.concatenate(
        [np.ascontiguousarray(results[c]["hout"].T).reshape(HC) for c in range(NCORES)]
    )
    out = h_new.reshape(1, 1, H).astype(np.float32)
    return out, out.copy()


def kernel(**inputs):
    in_maps = make_in_maps(inputs)
    res = run_on_hw(in_maps)
    return assemble(res.results)


# revision 29
# speedup vs baseline: 1.1866x; 1.0725x over previous
"""GRU cell (EncoderRNN single step) on 8 Trainium2 NeuronCores.

Full inputs -> full output. Sharding: each core owns a 256-wide slice of the
hidden dimension across all three gates (rows of w_ih/w_hh); no collectives.
The host gathers the embedding row (only that row of the table is needed) and
concatenates the 8 per-core h_new slices.

All matrix-vector work runs on the PE array via host-transposed k-major fp16
weights (measured LDWEIGHTS/MATMUL pair for [128,128]x[128,1]: ~27ns warm, so
the 384 pairs/core hide under the weight stream). DVE only does ten [128,2]
gate ops; ACT does the three sigmoids (tanh(v) = 2*sigmoid(2v)-1 keeps a
single act-table set).

Streaming: 6.29MB/core of fp16 weights over THREE DMA rows - the 16 SDMA
engines round-robin between rows at packet granularity, and a single row
leaves them ~50% idle on descriptor-packet fetch, so more rows = more
overlap. r rides first on the two HWDGE rings, z last on them, n on the
Pool SWDGE row (which starts earlier and lands mid-stream), so each gate's
sigmoid chain overlaps the next gate's stream.

Hard-won correctness rules (cold-run races otherwise, masked on reruns by
stale-but-identical SBUF/PSUM contents):
- ONE semaphore per DMA. A shared sem with >=16*(c+1) thresholds is unsound:
  the 16 SDMA engines' receipt increments all land in one counter, so a
  straggler engine can still be writing chunk c while faster engines push
  the sum past the threshold.
- The gate-complete sem must not fire before the last matmul's ~128-cycle
  PSUM drain lands: a dummy 192-column matmul (fence) carries the inc.
- start=True clears the has_written state for the whole 2KB PSUM bank
  region: only the FIRST matmul of each accumulator bank sets it.
"""

import sys

if "/opt/trn_rl_repo" not in sys.path:
    sys.path.insert(0, "/opt/trn_rl_repo")

import numpy as np
import ml_dtypes

H = 2048
NCORES = 8
HC = H // NCORES          # 256 hidden elems per core
UT = HC // 128            # 2 columns for the per-core [128, 2] gate slices
KC = H // 128             # 16 k-chunks
GMW = KC * HC             # 4096 cols per gate-matrix image ([128, 4096] f16)

_CACHE = {}


def _build():
    import contextlib
    from concourse import bacc, bass, mybir

    class _BareBlock(bass.BassBlock):
        # Skip the exit drains + all-engine EVSEM barrier: every cross-engine
        # dependency is semaphore-guarded and the issuing engine of the hout
        # DMA waits for its receipt, so nothing needs a terminal rendezvous.
        def __exit__(self, exc_type, exc_val, exc_tb):
            if exc_type is None:
                for engine, last_body in self.last_body.items():
                    with self.bass.body(
                        last_body, parent=self.bass.cur_bb, allow_existing_parent=True
                    ):
                        engine.br(self.end_bb)
                self.bass.switch_bb(self.end_bb)

    @contextlib.contextmanager
    def bare_block(nc):
        assert nc.cur_block is None
        with _BareBlock(nc, f"block_{nc.next_id()}") as blk:
            nc.cur_block = blk
            yield blk
        nc.cur_block = None

    f32 = mybir.dt.float32
    f16 = mybir.dt.float16
    Alu = mybir.AluOpType
    Act = mybir.ActivationFunctionType

    nc = bacc.Bacc(
        "TRN2",
        target_bir_lowering=False,
        debug=False,
        num_devices=NCORES,
        detect_race_conditions=False,
    )

    # one CONTIGUOUS 1MB DRAM tensor per gate-matrix slab: concatenating
    # slabs into a wider image makes every descriptor read 8KB then skip
    # 8KB, halving DRAM page locality (measured 16GB/s vs 25GB/s per queue)
    wri_d = nc.dram_tensor("wri", [128, GMW], f16, kind="ExternalInput")
    wrh_d = nc.dram_tensor("wrh", [128, GMW], f16, kind="ExternalInput")
    wni_d = nc.dram_tensor("wni", [128, GMW], f16, kind="ExternalInput")
    wnh_d = nc.dram_tensor("wnh", [128, GMW], f16, kind="ExternalInput")
    wzi_d = nc.dram_tensor("wzi", [128, GMW], f16, kind="ExternalInput")
    wzh_d = nc.dram_tensor("wzh", [128, GMW], f16, kind="ExternalInput")
    xc_d = nc.dram_tensor("xc", [128, KC], f16, kind="ExternalInput")
    hc_d = nc.dram_tensor("hc", [128, KC], f16, kind="ExternalInput")
    # cols: brz[0:2*UT], bin[2*UT:3*UT], bhn[3*UT:4*UT], hs[4*UT:5*UT]
    smalls = nc.dram_tensor("smalls", [128, 5 * UT], f32, kind="ExternalInput")
    hout = nc.dram_tensor("hout", [128, UT], f32, kind="ExternalOutput")

    sb = lambda name, shape, dt=f32: nc.alloc_sbuf_tensor(name, list(shape), dt).ap()
    wRi = sb("wRi", [128, GMW], f16)
    wRh = sb("wRh", [128, GMW], f16)
    wNi = sb("wNi", [128, GMW], f16)
    wNh = sb("wNh", [128, GMW], f16)
    wZi = sb("wZi", [128, GMW], f16)
    wZh = sb("wZh", [128, GMW], f16)
    xc = sb("xc_s", [128, KC], f16)
    hc = sb("hc_s", [128, KC], f16)
    sm = sb("sm", [128, 5 * UT])
    brz_t = sm[:, 0 : 2 * UT]
    bin_t = sm[:, 2 * UT : 3 * UT]
    bhn_t = sm[:, 3 * UT : 4 * UT]
    hs_t = sm[:, 4 * UT : 5 * UT]
    rp = sb("rp", [128, UT])
    zp = sb("zp", [128, UT])
    r_t = sb("r_t", [128, UT])
    z_t = sb("z_t", [128, UT])
    hnb = sb("hnb", [128, UT])
    t3 = sb("t3", [128, UT])
    t4 = sb("t4", [128, UT])
    s_tile = sb("s_tile", [128, UT])   # sigmoid(2v) for the n gate
    n_t = sb("n_t", [128, UT])         # u = 1 - 2s = -n
    t5 = sb("t5", [128, UT])
    hnew = sb("hnew", [128, UT])

    pe_fence = nc.alloc_psum_tensor("pe_fence", [128, 192], f32).ap()
    grp = nc.alloc_psum_tensor("grp", [128, UT], f32).ap()    # gi_r + gh_r
    gzp = nc.alloc_psum_tensor("gzp", [128, UT], f32).ap()    # gi_z + gh_z
    gin_p = nc.alloc_psum_tensor("gin_p", [128, UT], f32).ap()
    ghn_p = nc.alloc_psum_tensor("ghn_p", [128, UT], f32).ap()

    with contextlib.ExitStack() as _stack:
        sem = lambda n: _stack.enter_context(nc.semaphore(n))
        s_x = sem("s_x")
        s_h = sem("s_h")
        s_sm = sem("s_sm")
        s_wri = sem("s_wri")
        s_wrh = sem("s_wrh")
        s_wni = sem("s_wni")
        s_wnh = sem("s_wnh")
        s_wzi = sem("s_wzi")
        s_wzh = sem("s_wzh")
        s_gr = sem("s_gr")
        s_gn = sem("s_gn")
        s_gz = sem("s_gz")
        s_dve = sem("s_dve")
        s_act = sem("s_act")
        s_out = sem("s_out")
        block = _stack.enter_context(bare_block(nc))

        @block.sync
        def _(sync):
            sync.dma_start(out=xc[:, :], in_=xc_d.ap()[:, :]).then_inc(s_x, 16)
            sync.dma_start(out=sm[:, :], in_=smalls.ap()[:, :]).then_inc(s_sm, 16)
            sync.dma_start(out=wRi[:, :], in_=wri_d.ap()[:, :]).then_inc(s_wri, 16)
            sync.dma_start(out=wNi[:, :], in_=wni_d.ap()[:, :]).then_inc(s_wni, 16)
            sync.dma_start(out=wZi[:, :], in_=wzi_d.ap()[:, :]).then_inc(s_wzi, 16)

        @block.scalar
        def _(scalar):
            scalar.dma_start(out=hc[:, :], in_=hc_d.ap()[:, :]).then_inc(s_h, 16)
            scalar.dma_start(out=wRh[:, :], in_=wrh_d.ap()[:, :]).then_inc(s_wrh, 16)
            scalar.dma_start(out=wNh[:, :], in_=wnh_d.ap()[:, :]).then_inc(s_wnh, 16)
            scalar.dma_start(out=wZh[:, :], in_=wzh_d.ap()[:, :]).then_inc(s_wzh, 16)
            # r-gate sigmoid
            scalar.wait_ge(s_dve, 1)
            nc.scalar.activation(out=r_t[:, :], in_=rp[:, :], func=Act.Sigmoid).then_inc(
                s_act, 1
            )
            # n-gate tanh(v) = 2*sigmoid(2v) - 1, affine folded into DVE ops
            scalar.wait_ge(s_dve, 5)
            nc.scalar.activation(
                out=s_tile[:, :], in_=t4[:, :], func=Act.Sigmoid, scale=2.0
            ).then_inc(s_act, 1)
            # z-gate sigmoid
            scalar.wait_ge(s_dve, 8)
            nc.scalar.activation(out=z_t[:, :], in_=zp[:, :], func=Act.Sigmoid).then_inc(
                s_act, 1
            )
            scalar.wait_ge(s_dve, 10)
            scalar.dma_start(out=hout.ap()[:, :], in_=hnew[:, :]).then_inc(s_out, 16)
            scalar.wait_ge(s_out, 16)

        @block.tensor
        def _(tensor):
            def fence():
                return nc.tensor.matmul(
                    pe_fence[:, :],
                    lhsT=wRi[:, 0:128],
                    rhs=wRi[:, 0:192],
                    start=True,
                    stop=True,
                )

            def half_pairs(wt, vec, acc, start, stop):
                # 32 pairs: one gate-matrix slab [128, 16 k-tiles * 256]
                last = None
                for t in range(KC):
                    for j in range(UT):
                        last = nc.tensor.matmul(
                            acc[:, j : j + 1],
                            lhsT=wt[:, t * HC + j * 128 : t * HC + (j + 1) * 128],
                            rhs=vec[:, t : t + 1],
                            start=(start and t == 0 and j == 0),
                            stop=(stop and t == KC - 1),
                        )
                return last

            tensor.wait_ge(s_x, 16)
            tensor.wait_ge(s_h, 16)
            tensor.wait_ge(s_wri, 16)
            half_pairs(wRi, xc, grp, start=True, stop=False)
            tensor.wait_ge(s_wrh, 16)
            half_pairs(wRh, hc, grp, start=False, stop=True)
            fence().then_inc(s_gr, 1)
            tensor.wait_ge(s_wni, 16)
            half_pairs(wNi, xc, gin_p, start=True, stop=True)
            tensor.wait_ge(s_wnh, 16)
            half_pairs(wNh, hc, ghn_p, start=True, stop=True)
            fence().then_inc(s_gn, 1)
            tensor.wait_ge(s_wzi, 16)
            half_pairs(wZi, xc, gzp, start=True, stop=False)
            tensor.wait_ge(s_wzh, 16)
            half_pairs(wZh, hc, gzp, start=False, stop=True)
            fence().then_inc(s_gz, 1)

        @block.vector
        def _(vector):
            vector.wait_ge(s_gr, 1)
            vector.wait_ge(s_sm, 16)
            nc.vector.tensor_tensor(
                out=rp[:, :], in0=grp[:, :], in1=brz_t[:, 0:UT], op=Alu.add
            ).then_inc(s_dve, 1)  # 1 -> ACT sigmoid(r)
            vector.wait_ge(s_gn, 1)
            nc.vector.tensor_tensor(
                out=hnb[:, :], in0=ghn_p[:, :], in1=bhn_t[:, :], op=Alu.add
            ).then_inc(s_dve, 1)  # 2
            nc.vector.tensor_tensor(
                out=t4[:, :], in0=gin_p[:, :], in1=bin_t[:, :], op=Alu.add
            ).then_inc(s_dve, 1)  # 3
            vector.wait_ge(s_act, 1)
            vector.wait_ge(s_dve, 3)
            nc.vector.tensor_tensor(
                out=t3[:, :], in0=r_t[:, :], in1=hnb[:, :], op=Alu.mult
            ).then_inc(s_dve, 1)  # 4
            vector.wait_ge(s_dve, 4)
            nc.vector.tensor_tensor(
                out=t4[:, :], in0=t4[:, :], in1=t3[:, :], op=Alu.add
            ).then_inc(s_dve, 1)  # 5 -> ACT sigmoid(2v)
            vector.wait_ge(s_act, 2)
            # u = 1 - 2s = -n
            nc.vector.tensor_scalar(
                out=n_t[:, :], in0=s_tile[:, :], scalar1=-2.0, scalar2=1.0,
                op0=Alu.mult, op1=Alu.add,
            ).then_inc(s_dve, 1)  # 6
            vector.wait_ge(s_dve, 6)
            nc.vector.tensor_tensor(
                out=t5[:, :], in0=hs_t[:, :], in1=n_t[:, :], op=Alu.add
            ).then_inc(s_dve, 1)  # 7  (t5 = hs - n)
            vector.wait_ge(s_gz, 1)
            nc.vector.tensor_tensor(
                out=zp[:, :], in0=gzp[:, :], in1=brz_t[:, UT : 2 * UT], op=Alu.add
            ).then_inc(s_dve, 1)  # 8 -> ACT sigmoid(z)
            vector.wait_ge(s_act, 3)
            vector.wait_ge(s_dve, 7)
            nc.vector.tensor_tensor(
                out=t5[:, :], in0=z_t[:, :], in1=t5[:, :], op=Alu.mult
            ).then_inc(s_dve, 1)  # 9  (t5 = z * (hs - n))
            vector.wait_ge(s_dve, 9)
            nc.vector.tensor_tensor(
                out=hnew[:, :], in0=t5[:, :], in1=n_t[:, :], op=Alu.subtract
            ).then_inc(s_dve, 1)  # 10  (hnew = n + z*(hs - n))

    nc.compile()

    # Post-compile surgery:
    # 1. Strip the entry all-engine barrier (per-engine Drain + barrier_*
    #    EventSemaphores in the entry block). The only ordering it provides
    #    is Pool's preamble memsets vs other engines' const-AP reads; the
    #    first such read (ACT sigmoid bias) is ~15us after the ~1us memsets.
    blk0 = nc.main_func.blocks[0]
    kill = [
        i
        for i in blk0.instructions
        if isinstance(i, mybir.InstDrain)
        or (isinstance(i, mybir.InstEventSemaphore) and "barrier_" in str(i))
    ]
    assert len(kill) >= 10, f"expected entry barrier instrs, got {len(kill)}"
    for i in kill:
        blk0.instructions.remove(i)
    # 2. Move the entry LoadActFuncSet after the scalar-ring DMA issues so
    #    its 1.3us table load does not delay the ring start; the set that
    #    the sigmoids use loads right before them anyway.
    for b in nc.main_func.blocks:
        loads = [i for i in b.instructions if isinstance(i, mybir.InstLoadActFuncSet)]
        if len(loads) >= 2:
            first = loads[0]
            b.instructions.remove(first)
            dmas = [
                idx
                for idx, i in enumerate(b.instructions)
                if isinstance(i, mybir.InstDMACopy)
            ]
            b.instructions.insert(dmas[-1] + 1, first)
    return nc


def get_nc():
    if "nc" not in _CACHE:
        _CACHE["nc"] = _build()
    return _CACHE["nc"]


def make_in_maps(inputs):
    """Host-side sharding: full-input dict -> 8 per-core input maps."""
    emb = np.asarray(inputs["emb"], dtype=np.float32)
    w_ih = np.asarray(inputs["w_ih"], dtype=np.float32)
    w_hh = np.asarray(inputs["w_hh"], dtype=np.float32)
    b_ih = np.asarray(inputs["b_ih"], dtype=np.float32)
    b_hh = np.asarray(inputs["b_hh"], dtype=np.float32)
    idx = int(np.asarray(inputs["input"]).reshape(-1)[0])
    x = np.ascontiguousarray(emb[idx])
    h = np.asarray(inputs["hidden"], dtype=np.float32).reshape(H)

    xc_host = np.ascontiguousarray(x.reshape(KC, 128).T.astype(np.float16))
    hc_host = np.ascontiguousarray(h.reshape(KC, 128).T.astype(np.float16))
    bsum = b_ih + b_hh

    in_maps = []
    for c in range(NCORES):
        # per-core row slices, PyTorch gate order r, z, n
        sl = [slice(g * H + c * HC, g * H + c * HC + HC) for g in range(3)]
        r_sl, z_sl, n_sl = sl[0], sl[1], sl[2]

        # k-major gate-matrix image [128, 16*256]:
        # img[p, t*256 + o] = G^T[t*128 + p, o]
        def img(m, g_sl):
            rows = m[g_sl].T.astype(np.float16)          # [2048, 256]
            return rows.reshape(KC, 128, HC).transpose(1, 0, 2).reshape(128, GMW)

        slabs = {
            "wri": img(w_ih, r_sl), "wrh": img(w_hh, r_sl),
            "wni": img(w_ih, n_sl), "wnh": img(w_hh, n_sl),
            "wzi": img(w_ih, z_sl), "wzh": img(w_hh, z_sl),
        }
        slabs = {k: np.ascontiguousarray(v) for k, v in slabs.items()}
        brz_c = np.concatenate([bsum[r_sl], bsum[z_sl]]).reshape(2 * UT, 128).T
        bin_c = b_ih[n_sl].reshape(UT, 128).T
        bhn_c = b_hh[n_sl].reshape(UT, 128).T
        hs_c = h[c * HC : (c + 1) * HC].reshape(UT, 128).T
        smalls_c = np.ascontiguousarray(
            np.concatenate([brz_c, bin_c, bhn_c, hs_c], axis=1), dtype=np.float32
        )
        in_maps.append(
            {
                **slabs,
                "xc": xc_host,
                "hc": hc_host,
                "smalls": smalls_c,
            }
        )
    return in_maps


def run_on_hw(in_maps, trace=False):
    from concourse.bass_utils import run_bass_kernel_spmd

    kwargs = {}
    if trace:
        kwargs.update(trace=True, trace_cores=list(range(NCORES)))
    return run_bass_kernel_spmd(get_nc(), in_maps, core_ids=list(range(NCORES)), **kwargs)


def assemble(results):
    h_new = np.concatenate(
        [np.ascontiguousarray(results[c]["hout"].T).reshape(HC) for c in range(NCORES)]
    )
    out = h_new.reshape(1, 1, H).astype(np.float32)
    return out, out.copy()


def kernel(**inputs):
    in_maps = make_in_maps(inputs)
    res = run_on_hw(in_maps)
    return assemble(res.results)


# revision 31
# speedup vs baseline: 1.2892x; 1.0865x over previous
"""GRU cell (EncoderRNN single step) on 8 Trainium2 NeuronCores.

Full inputs -> full output. Sharding: each core owns a 256-wide slice of the
hidden dimension across all three gates (rows of w_ih/w_hh); no collectives.
The host gathers the embedding row (only that row of the table is needed) and
concatenates the 8 per-core h_new slices.

All matrix-vector work runs on the PE array via host-transposed k-major fp16
weights (measured LDWEIGHTS/MATMUL pair for [128,128]x[128,1]: ~27ns warm, so
the 384 pairs/core hide under the weight stream). DVE only does ten [128,2]
gate ops; ACT does the three sigmoids (tanh(v) = 2*sigmoid(2v)-1 keeps a
single act-table set).

Streaming: 6.29MB/core of fp16 weights over THREE DMA rows - the 16 SDMA
engines round-robin between rows at packet granularity, and a single row
leaves them ~50% idle on descriptor-packet fetch, so more rows = more
overlap. r rides first on the two HWDGE rings, z last on them, n on the
Pool SWDGE row (which starts earlier and lands mid-stream), so each gate's
sigmoid chain overlaps the next gate's stream.

Hard-won correctness rules (cold-run races otherwise, masked on reruns by
stale-but-identical SBUF/PSUM contents):
- ONE semaphore per DMA. A shared sem with >=16*(c+1) thresholds is unsound:
  the 16 SDMA engines' receipt increments all land in one counter, so a
  straggler engine can still be writing chunk c while faster engines push
  the sum past the threshold.
- The gate-complete sem must not fire before the last matmul's ~128-cycle
  PSUM drain lands: a dummy 192-column matmul (fence) carries the inc.
- start=True clears the has_written state for the whole 2KB PSUM bank
  region: only the FIRST matmul of each accumulator bank sets it.
"""

import sys

if "/opt/trn_rl_repo" not in sys.path:
    sys.path.insert(0, "/opt/trn_rl_repo")

import numpy as np
import ml_dtypes

H = 2048
NCORES = 8
HC = H // NCORES          # 256 hidden elems per core
UT = HC // 128            # 2 columns for the per-core [128, 2] gate slices
KC = H // 128             # 16 k-chunks
GMW = KC * HC             # 4096 cols per gate-matrix image ([128, 4096] f16)

_CACHE = {}


def _build():
    import contextlib
    from concourse import bacc, bass, mybir

    class _BareBlock(bass.BassBlock):
        # Skip the exit drains + all-engine EVSEM barrier: every cross-engine
        # dependency is semaphore-guarded and the issuing engine of the hout
        # DMA waits for its receipt, so nothing needs a terminal rendezvous.
        def __exit__(self, exc_type, exc_val, exc_tb):
            if exc_type is None:
                for engine, last_body in self.last_body.items():
                    with self.bass.body(
                        last_body, parent=self.bass.cur_bb, allow_existing_parent=True
                    ):
                        engine.br(self.end_bb)
                self.bass.switch_bb(self.end_bb)

    @contextlib.contextmanager
    def bare_block(nc):
        assert nc.cur_block is None
        with _BareBlock(nc, f"block_{nc.next_id()}") as blk:
            nc.cur_block = blk
            yield blk
        nc.cur_block = None

    f32 = mybir.dt.float32
    f16 = mybir.dt.float16
    Alu = mybir.AluOpType
    Act = mybir.ActivationFunctionType

    nc = bacc.Bacc(
        "TRN2",
        target_bir_lowering=False,
        debug=False,
        num_devices=NCORES,
        detect_race_conditions=False,
    )

    # one CONTIGUOUS DRAM tensor per half-slab (512KB, 4KB rows): wider
    # concatenated images make descriptors skip bytes and halve DRAM page
    # locality. 12 pieces, alternating rings, bound the damage from
    # run-to-run ring-rate skew to half a slab.
    # piece (g, m, k): gate g in (r,n,z), matrix m in (ih,hh), k-half
    wp_d = {
        (g, m, k): nc.dram_tensor(f"w{g}{m}{k}", [128, GMW // 2], f16,
                                  kind="ExternalInput")
        for g in "rnz" for m in "ih" for k in (0, 1)
    }
    ident_d = nc.dram_tensor("ident", [128, 128], f32, kind="ExternalInput")
    xc_d = nc.dram_tensor("xc", [128, KC], f16, kind="ExternalInput")
    hc_d = nc.dram_tensor("hc", [128, KC], f16, kind="ExternalInput")
    # cols: brz[0:2*UT], bin[2*UT:3*UT], bhn[3*UT:4*UT], hs[4*UT:5*UT]
    smalls = nc.dram_tensor("smalls", [128, 5 * UT], f32, kind="ExternalInput")
    hout = nc.dram_tensor("hout", [UT, 128], f32, kind="ExternalOutput")

    sb = lambda name, shape, dt=f32: nc.alloc_sbuf_tensor(name, list(shape), dt).ap()
    wp = {
        key: sb("s_w{}{}{}".format(*key), [128, GMW // 2], f16)
        for key in wp_d
    }
    ident = sb("ident_s", [128, 128], f32)
    houtT = sb("houtT", [2, 128])
    xc = sb("xc_s", [128, KC], f16)
    hc = sb("hc_s", [128, KC], f16)
    sm = sb("sm", [128, 5 * UT])
    brz_t = sm[:, 0 : 2 * UT]
    bin_t = sm[:, 2 * UT : 3 * UT]
    bhn_t = sm[:, 3 * UT : 4 * UT]
    hs_t = sm[:, 4 * UT : 5 * UT]
    rp = sb("rp", [128, UT])
    zp = sb("zp", [128, UT])
    r_t = sb("r_t", [128, UT])
    z_t = sb("z_t", [128, UT])
    hnb = sb("hnb", [128, UT])
    t3 = sb("t3", [128, UT])
    t4 = sb("t4", [128, UT])
    s_tile = sb("s_tile", [128, UT])   # sigmoid(2v) for the n gate
    n_t = sb("n_t", [128, UT])         # u = 1 - 2s = -n
    t5 = sb("t5", [128, UT])
    hnew = sb("hnew", [128, UT])

    pe_fence = nc.alloc_psum_tensor("pe_fence", [128, 192], f32).ap()
    hT_p = nc.alloc_psum_tensor("hT_p", [2, 128], f32).ap()
    grp = nc.alloc_psum_tensor("grp", [128, UT], f32).ap()    # gi_r + gh_r
    gzp = nc.alloc_psum_tensor("gzp", [128, UT], f32).ap()    # gi_z + gh_z
    gin_p = nc.alloc_psum_tensor("gin_p", [128, UT], f32).ap()
    ghn_p = nc.alloc_psum_tensor("ghn_p", [128, UT], f32).ap()

    with contextlib.ExitStack() as _stack:
        sem = lambda n: _stack.enter_context(nc.semaphore(n))
        s_x = sem("s_x")
        s_h = sem("s_h")
        s_sm = sem("s_sm")
        s_wp = {key: sem("sw{}{}{}".format(*key)) for key in wp_d}
        s_ident = sem("s_ident")
        s_hT = sem("s_hT")
        s_gr = sem("s_gr")
        s_gn = sem("s_gn")
        s_gz = sem("s_gz")
        s_dve = sem("s_dve")
        s_act = sem("s_act")
        s_out = sem("s_out")
        block = _stack.enter_context(bare_block(nc))

        SYNC_PIECES = [
            ("r", "i", 0), ("r", "h", 1), ("n", "i", 0),
            ("n", "h", 1), ("z", "i", 0), ("z", "h", 1),
        ]
        SCALAR_PIECES = [
            ("r", "h", 0), ("r", "i", 1), ("n", "h", 0),
            ("n", "i", 1), ("z", "h", 0), ("z", "i", 1),
        ]
        # PE consumption order (matches arrival: piece j of each ring lands
        # ~together; r first, z last)
        CONSUME = [
            ("r", "i", 0), ("r", "h", 0), ("r", "i", 1), ("r", "h", 1),
            ("n", "i", 0), ("n", "h", 0), ("n", "i", 1), ("n", "h", 1),
            ("z", "i", 0), ("z", "h", 0), ("z", "i", 1), ("z", "h", 1),
        ]

        @block.sync
        def _(sync):
            sync.dma_start(out=xc[:, :], in_=xc_d.ap()[:, :]).then_inc(s_x, 16)
            for key in SYNC_PIECES:
                sync.dma_start(out=wp[key][:, :], in_=wp_d[key].ap()[:, :]).then_inc(
                    s_wp[key], 16
                )
            sync.dma_start(out=sm[:, :], in_=smalls.ap()[:, :]).then_inc(s_sm, 16)
            sync.dma_start(out=ident[:, :], in_=ident_d.ap()[:, :]).then_inc(
                s_ident, 16
            )

        @block.scalar
        def _(scalar):
            scalar.dma_start(out=hc[:, :], in_=hc_d.ap()[:, :]).then_inc(s_h, 16)
            for key in SCALAR_PIECES:
                scalar.dma_start(out=wp[key][:, :], in_=wp_d[key].ap()[:, :]).then_inc(
                    s_wp[key], 16
                )
            # r-gate sigmoid
            scalar.wait_ge(s_dve, 1)
            nc.scalar.activation(out=r_t[:, :], in_=rp[:, :], func=Act.Sigmoid).then_inc(
                s_act, 1
            )
            # n-gate tanh(v) = 2*sigmoid(2v) - 1, affine folded into DVE ops
            scalar.wait_ge(s_dve, 5)
            nc.scalar.activation(
                out=s_tile[:, :], in_=t4[:, :], func=Act.Sigmoid, scale=2.0
            ).then_inc(s_act, 1)
            # z-gate sigmoid
            scalar.wait_ge(s_dve, 8)
            nc.scalar.activation(out=z_t[:, :], in_=zp[:, :], func=Act.Sigmoid).then_inc(
                s_act, 1
            )
            # hnew arrives PE-transposed in hT_p [2, 128]: copy to SBUF and
            # ship as 2 fat descriptors instead of 128x8B (whose per-engine
            # HBM write receipts spread over ~3us)
            scalar.wait_ge(s_hT, 1)
            nc.scalar.activation(
                out=houtT[:, :], in_=hT_p[:, :], func=Act.Copy
            ).then_inc(s_act, 1)  # 4
            scalar.wait_ge(s_act, 4)
            scalar.dma_start(out=hout.ap()[:, :], in_=houtT[:, :]).then_inc(s_out, 16)
            scalar.wait_ge(s_out, 16)

        @block.tensor
        def _(tensor):
            def fence():
                wr0 = wp[("r", "i", 0)]
                return nc.tensor.matmul(
                    pe_fence[:, :],
                    lhsT=wr0[:, 0:128],
                    rhs=wr0[:, 0:192],
                    start=True,
                    stop=True,
                )

            def piece_pairs(key, acc, start, stop):
                # 16 pairs: one half-slab [128, 8 k-tiles * 256]
                wt = wp[key]
                vec = xc if key[1] == "i" else hc
                koff = key[2] * (KC // 2)
                last = None
                for t in range(KC // 2):
                    for j in range(UT):
                        last = nc.tensor.matmul(
                            acc[:, j : j + 1],
                            lhsT=wt[:, t * HC + j * 128 : t * HC + (j + 1) * 128],
                            rhs=vec[:, koff + t : koff + t + 1],
                            start=(start and t == 0 and j == 0),
                            stop=(stop and t == KC // 2 - 1),
                        )
                return last

            tensor.wait_ge(s_x, 16)
            tensor.wait_ge(s_h, 16)
            accs = {"r": (grp, grp), "n": (gin_p, ghn_p), "z": (gzp, gzp)}
            fences = {"r": s_gr, "n": s_gn, "z": s_gz}
            for gi, g in enumerate("rnz"):
                acc_i, acc_h = accs[g]
                fused = acc_i is acc_h
                for key in [k for k in CONSUME if k[0] == g]:
                    acc = acc_i if key[1] == "i" else acc_h
                    first_i = key[1] == "i" and key[2] == 0
                    first_h = key[1] == "h" and key[2] == 0
                    last_i = key[1] == "i" and key[2] == 1
                    last_h = key[1] == "h" and key[2] == 1
                    tensor.wait_ge(s_wp[key], 16)
                    piece_pairs(
                        key,
                        acc,
                        start=first_i if fused else (first_i or first_h),
                        stop=last_h if fused else (last_i or last_h),
                    )
                fence().then_inc(fences[g], 1)
            # transpose hnew [128, 2] -> hT_p [2, 128] for the fat-descriptor
            # output DMA
            tensor.wait_ge(s_ident, 16)
            tensor.wait_ge(s_dve, 10)
            nc.tensor.transpose(hT_p[:, :], hnew[:, :], ident[:, :])
            fence().then_inc(s_hT, 1)

        @block.vector
        def _(vector):
            vector.wait_ge(s_gr, 1)
            vector.wait_ge(s_sm, 16)
            nc.vector.tensor_tensor(
                out=rp[:, :], in0=grp[:, :], in1=brz_t[:, 0:UT], op=Alu.add
            ).then_inc(s_dve, 1)  # 1 -> ACT sigmoid(r)
            vector.wait_ge(s_gn, 1)
            nc.vector.tensor_tensor(
                out=hnb[:, :], in0=ghn_p[:, :], in1=bhn_t[:, :], op=Alu.add
            ).then_inc(s_dve, 1)  # 2
            nc.vector.tensor_tensor(
                out=t4[:, :], in0=gin_p[:, :], in1=bin_t[:, :], op=Alu.add
            ).then_inc(s_dve, 1)  # 3
            vector.wait_ge(s_act, 1)
            vector.wait_ge(s_dve, 3)
            nc.vector.tensor_tensor(
                out=t3[:, :], in0=r_t[:, :], in1=hnb[:, :], op=Alu.mult
            ).then_inc(s_dve, 1)  # 4
            vector.wait_ge(s_dve, 4)
            nc.vector.tensor_tensor(
                out=t4[:, :], in0=t4[:, :], in1=t3[:, :], op=Alu.add
            ).then_inc(s_dve, 1)  # 5 -> ACT sigmoid(2v)
            vector.wait_ge(s_act, 2)
            # u = 1 - 2s = -n
            nc.vector.tensor_scalar(
                out=n_t[:, :], in0=s_tile[:, :], scalar1=-2.0, scalar2=1.0,
                op0=Alu.mult, op1=Alu.add,
            ).then_inc(s_dve, 1)  # 6
            vector.wait_ge(s_dve, 6)
            nc.vector.tensor_tensor(
                out=t5[:, :], in0=hs_t[:, :], in1=n_t[:, :], op=Alu.add
            ).then_inc(s_dve, 1)  # 7  (t5 = hs - n)
            vector.wait_ge(s_gz, 1)
            nc.vector.tensor_tensor(
                out=zp[:, :], in0=gzp[:, :], in1=brz_t[:, UT : 2 * UT], op=Alu.add
            ).then_inc(s_dve, 1)  # 8 -> ACT sigmoid(z)
            vector.wait_ge(s_act, 3)
            vector.wait_ge(s_dve, 7)
            nc.vector.tensor_tensor(
                out=t5[:, :], in0=z_t[:, :], in1=t5[:, :], op=Alu.mult
            ).then_inc(s_dve, 1)  # 9  (t5 = z * (hs - n))
            vector.wait_ge(s_dve, 9)
            nc.vector.tensor_tensor(
                out=hnew[:, :], in0=t5[:, :], in1=n_t[:, :], op=Alu.subtract
            ).then_inc(s_dve, 1)  # 10  (hnew = n + z*(hs - n))

    nc.compile()

    # Post-compile surgery:
    # 1. Strip the entry all-engine barrier (per-engine Drain + barrier_*
    #    EventSemaphores in the entry block). The only ordering it provides
    #    is Pool's preamble memsets vs other engines' const-AP reads; the
    #    first such read (ACT sigmoid bias) is ~15us after the ~1us memsets.
    blk0 = nc.main_func.blocks[0]
    kill = [
        i
        for i in blk0.instructions
        if isinstance(i, mybir.InstDrain)
        or (isinstance(i, mybir.InstEventSemaphore) and "barrier_" in str(i))
    ]
    assert len(kill) >= 10, f"expected entry barrier instrs, got {len(kill)}"
    for i in kill:
        blk0.instructions.remove(i)
    # 2. Move the entry LoadActFuncSet after the scalar-ring DMA issues so
    #    its 1.3us table load does not delay the ring start; the set that
    #    the sigmoids use loads right before them anyway.
    for b in nc.main_func.blocks:
        loads = [i for i in b.instructions if isinstance(i, mybir.InstLoadActFuncSet)]
        if len(loads) >= 2:
            first = loads[0]
            b.instructions.remove(first)
            dmas = [
                idx
                for idx, i in enumerate(b.instructions)
                if isinstance(i, mybir.InstDMACopy)
            ]
            b.instructions.insert(dmas[-1] + 1, first)
    return nc


def get_nc():
    if "nc" not in _CACHE:
        _CACHE["nc"] = _build()
    return _CACHE["nc"]


def make_in_maps(inputs):
    """Host-side sharding: full-input dict -> 8 per-core input maps."""
    emb = np.asarray(inputs["emb"], dtype=np.float32)
    w_ih = np.asarray(inputs["w_ih"], dtype=np.float32)
    w_hh = np.asarray(inputs["w_hh"], dtype=np.float32)
    b_ih = np.asarray(inputs["b_ih"], dtype=np.float32)
    b_hh = np.asarray(inputs["b_hh"], dtype=np.float32)
    idx = int(np.asarray(inputs["input"]).reshape(-1)[0])
    x = np.ascontiguousarray(emb[idx])
    h = np.asarray(inputs["hidden"], dtype=np.float32).reshape(H)

    xc_host = np.ascontiguousarray(x.reshape(KC, 128).T.astype(np.float16))
    hc_host = np.ascontiguousarray(h.reshape(KC, 128).T.astype(np.float16))
    bsum = b_ih + b_hh

    in_maps = []
    for c in range(NCORES):
        # per-core row slices, PyTorch gate order r, z, n
        sl = [slice(g * H + c * HC, g * H + c * HC + HC) for g in range(3)]
        r_sl, z_sl, n_sl = sl[0], sl[1], sl[2]

        # k-major gate-matrix image [128, 16*256]:
        # img[p, t*256 + o] = G^T[t*128 + p, o]
        def img(m, g_sl):
            rows = m[g_sl].T.astype(np.float16)          # [2048, 256]
            return rows.reshape(KC, 128, HC).transpose(1, 0, 2).reshape(128, GMW)

        half = GMW // 2
        slabs = {}
        for g, g_slc in (("r", r_sl), ("n", n_sl), ("z", z_sl)):
            for m, mat in (("i", w_ih), ("h", w_hh)):
                full = img(mat, g_slc)
                slabs[f"w{g}{m}0"] = np.ascontiguousarray(full[:, :half])
                slabs[f"w{g}{m}1"] = np.ascontiguousarray(full[:, half:])
        brz_c = np.concatenate([bsum[r_sl], bsum[z_sl]]).reshape(2 * UT, 128).T
        bin_c = b_ih[n_sl].reshape(UT, 128).T
        bhn_c = b_hh[n_sl].reshape(UT, 128).T
        hs_c = h[c * HC : (c + 1) * HC].reshape(UT, 128).T
        smalls_c = np.ascontiguousarray(
            np.concatenate([brz_c, bin_c, bhn_c, hs_c], axis=1), dtype=np.float32
        )
        in_maps.append(
            {
                **slabs,
                "xc": xc_host,
                "hc": hc_host,
                "smalls": smalls_c,
                "ident": np.eye(128, dtype=np.float32),
            }
        )
    return in_maps


def run_on_hw(in_maps, trace=False):
    from concourse.bass_utils import run_bass_kernel_spmd

    kwargs = {}
    if trace:
        kwargs.update(trace=True, trace_cores=list(range(NCORES)))
    return run_bass_kernel_spmd(get_nc(), in_maps, core_ids=list(range(NCORES)), **kwargs)


def assemble(results):
    h_new = np.concatenate(
        [np.ascontiguousarray(results[c]["hout"]).reshape(HC) for c in range(NCORES)]
    )
    out = h_new.reshape(1, 1, H).astype(np.float32)
    return out, out.copy()


def kernel(**inputs):
    in_maps = make_in_maps(inputs)
    res = run_on_hw(in_maps)
    return assemble(res.results)


# revision 33
# speedup vs baseline: 1.3016x; 1.0096x over previous
"""GRU cell (EncoderRNN single step) on 8 Trainium2 NeuronCores.

Full inputs -> full output. Sharding: each core owns a 256-wide slice of the
hidden dimension across all three gates (rows of w_ih/w_hh); no collectives.
The host gathers the embedding row (only that row of the table is needed) and
concatenates the 8 per-core h_new slices.

All matrix-vector work runs on the PE array via host-transposed k-major fp16
weights (measured LDWEIGHTS/MATMUL pair for [128,128]x[128,1]: ~27ns warm, so
the 384 pairs/core hide under the weight stream). DVE only does ten [128,2]
gate ops; ACT does the three sigmoids (tanh(v) = 2*sigmoid(2v)-1 keeps a
single act-table set).

Streaming: 6.29MB/core of fp16 weights over THREE DMA rows - the 16 SDMA
engines round-robin between rows at packet granularity, and a single row
leaves them ~50% idle on descriptor-packet fetch, so more rows = more
overlap. r rides first on the two HWDGE rings, z last on them, n on the
Pool SWDGE row (which starts earlier and lands mid-stream), so each gate's
sigmoid chain overlaps the next gate's stream.

Hard-won correctness rules (cold-run races otherwise, masked on reruns by
stale-but-identical SBUF/PSUM contents):
- ONE semaphore per DMA. A shared sem with >=16*(c+1) thresholds is unsound:
  the 16 SDMA engines' receipt increments all land in one counter, so a
  straggler engine can still be writing chunk c while faster engines push
  the sum past the threshold.
- The gate-complete sem must not fire before the last matmul's ~128-cycle
  PSUM drain lands: a dummy 192-column matmul (fence) carries the inc.
- start=True clears the has_written state for the whole 2KB PSUM bank
  region: only the FIRST matmul of each accumulator bank sets it.
"""

import sys

if "/opt/trn_rl_repo" not in sys.path:
    sys.path.insert(0, "/opt/trn_rl_repo")

import numpy as np
import ml_dtypes

H = 2048
NCORES = 8
HC = H // NCORES          # 256 hidden elems per core
UT = HC // 128            # 2 columns for the per-core [128, 2] gate slices
KC = H // 128             # 16 k-chunks
GMW = KC * HC             # 4096 cols per gate-matrix image ([128, 4096] f16)

_CACHE = {}


def _build():
    import contextlib
    from concourse import bacc, bass, mybir

    class _BareBlock(bass.BassBlock):
        # Skip the exit drains + all-engine EVSEM barrier: every cross-engine
        # dependency is semaphore-guarded and the issuing engine of the hout
        # DMA waits for its receipt, so nothing needs a terminal rendezvous.
        def __exit__(self, exc_type, exc_val, exc_tb):
            if exc_type is None:
                for engine, last_body in self.last_body.items():
                    with self.bass.body(
                        last_body, parent=self.bass.cur_bb, allow_existing_parent=True
                    ):
                        engine.br(self.end_bb)
                self.bass.switch_bb(self.end_bb)

    @contextlib.contextmanager
    def bare_block(nc):
        assert nc.cur_block is None
        with _BareBlock(nc, f"block_{nc.next_id()}") as blk:
            nc.cur_block = blk
            yield blk
        nc.cur_block = None

    f32 = mybir.dt.float32
    f16 = mybir.dt.float16
    Alu = mybir.AluOpType
    Act = mybir.ActivationFunctionType

    nc = bacc.Bacc(
        "TRN2",
        target_bir_lowering=False,
        debug=False,
        num_devices=NCORES,
        detect_race_conditions=False,
    )

    # one CONTIGUOUS DRAM tensor per half-slab (512KB, 4KB rows): wider
    # concatenated images make descriptors skip bytes and halve DRAM page
    # locality. 12 pieces, alternating rings, bound the damage from
    # run-to-run ring-rate skew to half a slab.
    # piece (g, m, k): gate g in (r,n,z), matrix m in (ih,hh), k-half
    wp_d = {
        (g, m, k): nc.dram_tensor(f"w{g}{m}{k}", [128, GMW // 2], f16,
                                  kind="ExternalInput")
        for g in "rnz" for m in "ih" for k in (0, 1)
    }
    ident_d = nc.dram_tensor("ident", [128, 128], f32, kind="ExternalInput")
    xc_d = nc.dram_tensor("xc", [128, KC], f16, kind="ExternalInput")
    hc_d = nc.dram_tensor("hc", [128, KC], f16, kind="ExternalInput")
    # cols: brz[0:2*UT], bin[2*UT:3*UT], bhn[3*UT:4*UT], hs[4*UT:5*UT]
    smalls = nc.dram_tensor("smalls", [128, 5 * UT], f32, kind="ExternalInput")
    hout = nc.dram_tensor("hout", [UT, 128], f32, kind="ExternalOutput")

    sb = lambda name, shape, dt=f32: nc.alloc_sbuf_tensor(name, list(shape), dt).ap()
    wp = {
        key: sb("s_w{}{}{}".format(*key), [128, GMW // 2], f16)
        for key in wp_d
    }
    ident = sb("ident_s", [128, 128], f32)
    houtT = sb("houtT", [2, 128])
    xc = sb("xc_s", [128, KC], f16)
    hc = sb("hc_s", [128, KC], f16)
    sm = sb("sm", [128, 5 * UT])
    brz_t = sm[:, 0 : 2 * UT]
    bin_t = sm[:, 2 * UT : 3 * UT]
    bhn_t = sm[:, 3 * UT : 4 * UT]
    hs_t = sm[:, 4 * UT : 5 * UT]
    rp = sb("rp", [128, UT])
    zp = sb("zp", [128, UT])
    r_t = sb("r_t", [128, UT])
    z_t = sb("z_t", [128, UT])
    hnb = sb("hnb", [128, UT])
    t3 = sb("t3", [128, UT])
    t4 = sb("t4", [128, UT])
    s_tile = sb("s_tile", [128, UT])   # sigmoid(2v) for the n gate
    n_t = sb("n_t", [128, UT])         # u = 1 - 2s = -n
    t5 = sb("t5", [128, UT])
    hnew = sb("hnew", [128, UT])

    pe_fence = nc.alloc_psum_tensor("pe_fence", [128, 192], f32).ap()
    hT_p = nc.alloc_psum_tensor("hT_p", [2, 128], f32).ap()
    grp = nc.alloc_psum_tensor("grp", [128, UT], f32).ap()    # gi_r + gh_r
    gzp = nc.alloc_psum_tensor("gzp", [128, UT], f32).ap()    # gi_z + gh_z
    gin_p = nc.alloc_psum_tensor("gin_p", [128, UT], f32).ap()
    ghn_p = nc.alloc_psum_tensor("ghn_p", [128, UT], f32).ap()

    with contextlib.ExitStack() as _stack:
        sem = lambda n: _stack.enter_context(nc.semaphore(n))
        s_x = sem("s_x")
        s_h = sem("s_h")
        s_sm = sem("s_sm")
        s_wp = {key: sem("sw{}{}{}".format(*key)) for key in wp_d}
        s_ident = sem("s_ident")
        s_hT = sem("s_hT")
        s_gr = sem("s_gr")
        s_gn = sem("s_gn")
        s_gz = sem("s_gz")
        s_dve = sem("s_dve")
        s_act = sem("s_act")
        s_out = sem("s_out")
        block = _stack.enter_context(bare_block(nc))

        SYNC_PIECES = [
            ("r", "i", 0), ("r", "h", 1), ("n", "i", 0),
            ("n", "h", 1), ("z", "i", 0), ("z", "h", 1),
        ]
        SCALAR_PIECES = [
            ("r", "h", 0), ("r", "i", 1), ("n", "h", 0),
            ("n", "i", 1), ("z", "h", 0), ("z", "i", 1),
        ]
        # PE consumption order (matches arrival: piece j of each ring lands
        # ~together; r first, z last)
        CONSUME = [
            ("r", "i", 0), ("r", "h", 0), ("r", "i", 1), ("r", "h", 1),
            ("n", "i", 0), ("n", "h", 0), ("n", "i", 1), ("n", "h", 1),
            ("z", "i", 0), ("z", "h", 0), ("z", "i", 1), ("z", "h", 1),
        ]

        @block.sync
        def _(sync):
            sync.dma_start(out=xc[:, :], in_=xc_d.ap()[:, :]).then_inc(s_x, 16)
            sync.dma_start(out=sm[:, :], in_=smalls.ap()[:, :]).then_inc(s_sm, 16)
            for key in SYNC_PIECES:
                sync.dma_start(out=wp[key][:, :], in_=wp_d[key].ap()[:, :]).then_inc(
                    s_wp[key], 16
                )
            sync.dma_start(out=ident[:, :], in_=ident_d.ap()[:, :]).then_inc(
                s_ident, 16
            )

        @block.scalar
        def _(scalar):
            scalar.dma_start(out=hc[:, :], in_=hc_d.ap()[:, :]).then_inc(s_h, 16)
            for key in SCALAR_PIECES:
                scalar.dma_start(out=wp[key][:, :], in_=wp_d[key].ap()[:, :]).then_inc(
                    s_wp[key], 16
                )
            # r-gate sigmoid
            scalar.wait_ge(s_dve, 1)
            nc.scalar.activation(out=r_t[:, :], in_=rp[:, :], func=Act.Sigmoid).then_inc(
                s_act, 1
            )
            # n-gate tanh(v) = 2*sigmoid(2v) - 1, affine folded into DVE ops
            scalar.wait_ge(s_dve, 5)
            nc.scalar.activation(
                out=s_tile[:, :], in_=t4[:, :], func=Act.Sigmoid, scale=2.0
            ).then_inc(s_act, 1)
            # z-gate sigmoid
            scalar.wait_ge(s_dve, 8)
            nc.scalar.activation(out=z_t[:, :], in_=zp[:, :], func=Act.Sigmoid).then_inc(
                s_act, 1
            )
            # hnew arrives PE-transposed in hT_p [2, 128]: copy to SBUF and
            # ship as 2 fat descriptors instead of 128x8B (whose per-engine
            # HBM write receipts spread over ~3us)
            scalar.wait_ge(s_hT, 1)
            nc.scalar.activation(
                out=houtT[:, :], in_=hT_p[:, :], func=Act.Copy
            ).then_inc(s_act, 1)  # 4
            scalar.wait_ge(s_act, 4)
            scalar.dma_start(out=hout.ap()[:, :], in_=houtT[:, :]).then_inc(s_out, 16)
            scalar.wait_ge(s_out, 16)

        @block.tensor
        def _(tensor):
            def fence():
                wr0 = wp[("r", "i", 0)]
                return nc.tensor.matmul(
                    pe_fence[:, :],
                    lhsT=wr0[:, 0:128],
                    rhs=wr0[:, 0:192],
                    start=True,
                    stop=True,
                )

            def piece_pairs(key, acc, start, stop):
                # 16 pairs: one half-slab [128, 8 k-tiles * 256]
                wt = wp[key]
                vec = xc if key[1] == "i" else hc
                koff = key[2] * (KC // 2)
                last = None
                for t in range(KC // 2):
                    for j in range(UT):
                        last = nc.tensor.matmul(
                            acc[:, j : j + 1],
                            lhsT=wt[:, t * HC + j * 128 : t * HC + (j + 1) * 128],
                            rhs=vec[:, koff + t : koff + t + 1],
                            start=(start and t == 0 and j == 0),
                            stop=(stop and t == KC // 2 - 1),
                        )
                return last

            tensor.wait_ge(s_x, 16)
            tensor.wait_ge(s_h, 16)
            accs = {"r": (grp, grp), "n": (gin_p, ghn_p), "z": (gzp, gzp)}
            fences = {"r": s_gr, "n": s_gn, "z": s_gz}
            for gi, g in enumerate("rnz"):
                acc_i, acc_h = accs[g]
                fused = acc_i is acc_h
                for key in [k for k in CONSUME if k[0] == g]:
                    acc = acc_i if key[1] == "i" else acc_h
                    first_i = key[1] == "i" and key[2] == 0
                    first_h = key[1] == "h" and key[2] == 0
                    last_i = key[1] == "i" and key[2] == 1
                    last_h = key[1] == "h" and key[2] == 1
                    tensor.wait_ge(s_wp[key], 16)
                    piece_pairs(
                        key,
                        acc,
                        start=first_i if fused else (first_i or first_h),
                        stop=last_h if fused else (last_i or last_h),
                    )
                fence().then_inc(fences[g], 1)
            # transpose hnew [128, 2] -> hT_p [2, 128] for the fat-descriptor
            # output DMA
            tensor.wait_ge(s_ident, 16)
            tensor.wait_ge(s_dve, 10)
            nc.tensor.transpose(hT_p[:, :], hnew[:, :], ident[:, :])
            fence().then_inc(s_hT, 1)

        @block.vector
        def _(vector):
            vector.wait_ge(s_gr, 1)
            vector.wait_ge(s_sm, 16)
            nc.vector.tensor_tensor(
                out=rp[:, :], in0=grp[:, :], in1=brz_t[:, 0:UT], op=Alu.add
            ).then_inc(s_dve, 1)  # 1 -> ACT sigmoid(r)
            vector.wait_ge(s_gn, 1)
            nc.vector.tensor_tensor(
                out=hnb[:, :], in0=ghn_p[:, :], in1=bhn_t[:, :], op=Alu.add
            ).then_inc(s_dve, 1)  # 2
            nc.vector.tensor_tensor(
                out=t4[:, :], in0=gin_p[:, :], in1=bin_t[:, :], op=Alu.add
            ).then_inc(s_dve, 1)  # 3
            vector.wait_ge(s_act, 1)
            vector.wait_ge(s_dve, 3)
            nc.vector.tensor_tensor(
                out=t3[:, :], in0=r_t[:, :], in1=hnb[:, :], op=Alu.mult
            ).then_inc(s_dve, 1)  # 4
            vector.wait_ge(s_dve, 4)
            nc.vector.tensor_tensor(
                out=t4[:, :], in0=t4[:, :], in1=t3[:, :], op=Alu.add
            ).then_inc(s_dve, 1)  # 5 -> ACT sigmoid(2v)
            vector.wait_ge(s_act, 2)
            # u = 1 - 2s = -n
            nc.vector.tensor_scalar(
                out=n_t[:, :], in0=s_tile[:, :], scalar1=-2.0, scalar2=1.0,
                op0=Alu.mult, op1=Alu.add,
            ).then_inc(s_dve, 1)  # 6
            vector.wait_ge(s_dve, 6)
            nc.vector.tensor_tensor(
                out=t5[:, :], in0=hs_t[:, :], in1=n_t[:, :], op=Alu.add
            ).then_inc(s_dve, 1)  # 7  (t5 = hs - n)
            vector.wait_ge(s_gz, 1)
            nc.vector.tensor_tensor(
                out=zp[:, :], in0=gzp[:, :], in1=brz_t[:, UT : 2 * UT], op=Alu.add
            ).then_inc(s_dve, 1)  # 8 -> ACT sigmoid(z)
            vector.wait_ge(s_act, 3)
            vector.wait_ge(s_dve, 7)
            nc.vector.tensor_tensor(
                out=t5[:, :], in0=z_t[:, :], in1=t5[:, :], op=Alu.mult
            ).then_inc(s_dve, 1)  # 9  (t5 = z * (hs - n))
            vector.wait_ge(s_dve, 9)
            nc.vector.tensor_tensor(
                out=hnew[:, :], in0=t5[:, :], in1=n_t[:, :], op=Alu.subtract
            ).then_inc(s_dve, 1)  # 10  (hnew = n + z*(hs - n))

    nc.compile()

    # Post-compile surgery:
    # 1. Strip the entry all-engine barrier (per-engine Drain + barrier_*
    #    EventSemaphores in the entry block). The only ordering it provides
    #    is Pool's preamble memsets vs other engines' const-AP reads; the
    #    first such read (ACT sigmoid bias) is ~15us after the ~1us memsets.
    blk0 = nc.main_func.blocks[0]
    kill = [
        i
        for i in blk0.instructions
        if isinstance(i, mybir.InstDrain)
        or (isinstance(i, mybir.InstEventSemaphore) and "barrier_" in str(i))
    ]
    assert len(kill) >= 10, f"expected entry barrier instrs, got {len(kill)}"
    for i in kill:
        blk0.instructions.remove(i)
    # 2. Move the entry LoadActFuncSet after the scalar-ring DMA issues so
    #    its 1.3us table load does not delay the ring start; the set that
    #    the sigmoids use loads right before them anyway.
    for b in nc.main_func.blocks:
        loads = [i for i in b.instructions if isinstance(i, mybir.InstLoadActFuncSet)]
        if len(loads) >= 2:
            first = loads[0]
            b.instructions.remove(first)
            acts = [
                idx
                for idx, i in enumerate(b.instructions)
                if isinstance(i, (mybir.InstActivation, mybir.InstLoadActFuncSet))
            ]
            b.instructions.insert(acts[0], first)
    return nc


def get_nc():
    if "nc" not in _CACHE:
        _CACHE["nc"] = _build()
    return _CACHE["nc"]


def make_in_maps(inputs):
    """Host-side sharding: full-input dict -> 8 per-core input maps."""
    emb = np.asarray(inputs["emb"], dtype=np.float32)
    w_ih = np.asarray(inputs["w_ih"], dtype=np.float32)
    w_hh = np.asarray(inputs["w_hh"], dtype=np.float32)
    b_ih = np.asarray(inputs["b_ih"], dtype=np.float32)
    b_hh = np.asarray(inputs["b_hh"], dtype=np.float32)
    idx = int(np.asarray(inputs["input"]).reshape(-1)[0])
    x = np.ascontiguousarray(emb[idx])
    h = np.asarray(inputs["hidden"], dtype=np.float32).reshape(H)

    xc_host = np.ascontiguousarray(x.reshape(KC, 128).T.astype(np.float16))
    hc_host = np.ascontiguousarray(h.reshape(KC, 128).T.astype(np.float16))
    bsum = b_ih + b_hh

    in_maps = []
    for c in range(NCORES):
        # per-core row slices, PyTorch gate order r, z, n
        sl = [slice(g * H + c * HC, g * H + c * HC + HC) for g in range(3)]
        r_sl, z_sl, n_sl = sl[0], sl[1], sl[2]

        # k-major gate-matrix image [128, 16*256]:
        # img[p, t*256 + o] = G^T[t*128 + p, o]
        def img(m, g_sl):
            rows = m[g_sl].T.astype(np.float16)          # [2048, 256]
            return rows.reshape(KC, 128, HC).transpose(1, 0, 2).reshape(128, GMW)

        half = GMW // 2
        slabs = {}
        for g, g_slc in (("r", r_sl), ("n", n_sl), ("z", z_sl)):
            for m, mat in (("i", w_ih), ("h", w_hh)):
                full = img(mat, g_slc)
                slabs[f"w{g}{m}0"] = np.ascontiguousarray(full[:, :half])
                slabs[f"w{g}{m}1"] = np.ascontiguousarray(full[:, half:])
        brz_c = np.concatenate([bsum[r_sl], bsum[z_sl]]).reshape(2 * UT, 128).T
        bin_c = b_ih[n_sl].reshape(UT, 128).T
        bhn_c = b_hh[n_sl].reshape(UT, 128).T
        hs_c = h[c * HC : (c + 1) * HC].reshape(UT, 128).T
        smalls_c = np.ascontiguousarray(
            np.concatenate([brz_c, bin_c, bhn_c, hs_c], axis=1), dtype=np.float32
        )
        in_maps.append(
            {
                **slabs,
                "xc": xc_host,
                "hc": hc_host,
                "smalls": smalls_c,
                "ident": np.eye(128, dtype=np.float32),
            }
        )
    return in_maps


def run_on_hw(in_maps, trace=False):
    from concourse.bass_utils import run_bass_kernel_spmd

    kwargs = {}
    if trace:
        kwargs.update(trace=True, trace_cores=list(range(NCORES)))
    return run_bass_kernel_spmd(get_nc(), in_maps, core_ids=list(range(NCORES)), **kwargs)


def assemble(results):
    h_new = np.concatenate(
        [np.ascontiguousarray(results[c]["hout"]).reshape(HC) for c in range(NCORES)]
    )
    out = h_new.reshape(1, 1, H).astype(np.float32)
    return out, out.copy()


def kernel(**inputs):
    in_maps = make_in_maps(inputs)
    res = run_on_hw(in_maps)
    return assemble(res.results)


# revision 34
# speedup vs baseline: 1.3037x; 1.0016x over previous
"""GRU cell (EncoderRNN single step) on 8 Trainium2 NeuronCores.

Full inputs -> full output. Sharding: each core owns a 256-wide slice of the
hidden dimension across all three gates (rows of w_ih/w_hh); no collectives.
The host gathers the embedding row (only that row of the table is needed) and
concatenates the 8 per-core h_new slices.

All matrix-vector work runs on the PE array via host-transposed k-major fp16
weights (measured LDWEIGHTS/MATMUL pair for [128,128]x[128,1]: ~27ns warm, so
the 384 pairs/core hide under the weight stream). DVE only does ten [128,2]
gate ops; ACT does the three sigmoids (tanh(v) = 2*sigmoid(2v)-1 keeps a
single act-table set).

Streaming: 6.29MB/core of fp16 weights over THREE DMA rows - the 16 SDMA
engines round-robin between rows at packet granularity, and a single row
leaves them ~50% idle on descriptor-packet fetch, so more rows = more
overlap. r rides first on the two HWDGE rings, z last on them, n on the
Pool SWDGE row (which starts earlier and lands mid-stream), so each gate's
sigmoid chain overlaps the next gate's stream.

Hard-won correctness rules (cold-run races otherwise, masked on reruns by
stale-but-identical SBUF/PSUM contents):
- ONE semaphore per DMA. A shared sem with >=16*(c+1) thresholds is unsound:
  the 16 SDMA engines' receipt increments all land in one counter, so a
  straggler engine can still be writing chunk c while faster engines push
  the sum past the threshold.
- The gate-complete sem must not fire before the last matmul's ~128-cycle
  PSUM drain lands: a dummy 192-column matmul (fence) carries the inc.
- start=True clears the has_written state for the whole 2KB PSUM bank
  region: only the FIRST matmul of each accumulator bank sets it.
"""

import sys

if "/opt/trn_rl_repo" not in sys.path:
    sys.path.insert(0, "/opt/trn_rl_repo")

import numpy as np
import ml_dtypes

H = 2048
NCORES = 8
HC = H // NCORES          # 256 hidden elems per core
UT = HC // 128            # 2 columns for the per-core [128, 2] gate slices
KC = H // 128             # 16 k-chunks
GMW = KC * HC             # 4096 cols per gate-matrix image ([128, 4096] f16)

_CACHE = {}


def _build():
    import contextlib
    from concourse import bacc, bass, mybir

    class _BareBlock(bass.BassBlock):
        # Skip the exit drains + all-engine EVSEM barrier: every cross-engine
        # dependency is semaphore-guarded and the issuing engine of the hout
        # DMA waits for its receipt, so nothing needs a terminal rendezvous.
        def __exit__(self, exc_type, exc_val, exc_tb):
            if exc_type is None:
                for engine, last_body in self.last_body.items():
                    with self.bass.body(
                        last_body, parent=self.bass.cur_bb, allow_existing_parent=True
                    ):
                        engine.br(self.end_bb)
                self.bass.switch_bb(self.end_bb)

    @contextlib.contextmanager
    def bare_block(nc):
        assert nc.cur_block is None
        with _BareBlock(nc, f"block_{nc.next_id()}") as blk:
            nc.cur_block = blk
            yield blk
        nc.cur_block = None

    f32 = mybir.dt.float32
    f16 = mybir.dt.float16
    Alu = mybir.AluOpType
    Act = mybir.ActivationFunctionType

    nc = bacc.Bacc(
        "TRN2",
        target_bir_lowering=False,
        debug=False,
        num_devices=NCORES,
        detect_race_conditions=False,
    )

    # one CONTIGUOUS DRAM tensor per half-slab (512KB, 4KB rows): wider
    # concatenated images make descriptors skip bytes and halve DRAM page
    # locality. 12 pieces, alternating rings, bound the damage from
    # run-to-run ring-rate skew to half a slab.
    # piece (g, m, k): gate g in (r,n,z), matrix m in (ih,hh), k-half
    # r/n in 512KB halves; z in 256KB quarters so the last-arriving piece
    # only gates ~8 PE pairs of tail work
    def _pieces():
        for g in "rn":
            for m in "ih":
                for k in (0, 1):
                    yield (g, m, k), GMW // 2
        for m in "ih":
            for k in (0, 1, 2, 3):
                yield ("z", m, k), GMW // 4

    PIECES = dict(_pieces())
    wp_d = {
        key: nc.dram_tensor("w{}{}{}".format(*key), [128, w], f16,
                            kind="ExternalInput")
        for key, w in PIECES.items()
    }
    ident_d = nc.dram_tensor("ident", [128, 128], f32, kind="ExternalInput")
    xc_d = nc.dram_tensor("xc", [128, KC], f16, kind="ExternalInput")
    hc_d = nc.dram_tensor("hc", [128, KC], f16, kind="ExternalInput")
    # cols: brz[0:2*UT], bin[2*UT:3*UT], bhn[3*UT:4*UT], hs[4*UT:5*UT]
    smalls = nc.dram_tensor("smalls", [128, 5 * UT], f32, kind="ExternalInput")
    hout = nc.dram_tensor("hout", [UT, 128], f32, kind="ExternalOutput")

    sb = lambda name, shape, dt=f32: nc.alloc_sbuf_tensor(name, list(shape), dt).ap()
    wp = {
        key: sb("s_w{}{}{}".format(*key), [128, PIECES[key]], f16)
        for key in wp_d
    }
    ident = sb("ident_s", [128, 128], f32)
    houtT = sb("houtT", [2, 128])
    xc = sb("xc_s", [128, KC], f16)
    hc = sb("hc_s", [128, KC], f16)
    sm = sb("sm", [128, 5 * UT])
    brz_t = sm[:, 0 : 2 * UT]
    bin_t = sm[:, 2 * UT : 3 * UT]
    bhn_t = sm[:, 3 * UT : 4 * UT]
    hs_t = sm[:, 4 * UT : 5 * UT]
    rp = sb("rp", [128, UT])
    zp = sb("zp", [128, UT])
    r_t = sb("r_t", [128, UT])
    z_t = sb("z_t", [128, UT])
    hnb = sb("hnb", [128, UT])
    t3 = sb("t3", [128, UT])
    t4 = sb("t4", [128, UT])
    s_tile = sb("s_tile", [128, UT])   # sigmoid(2v) for the n gate
    n_t = sb("n_t", [128, UT])         # u = 1 - 2s = -n
    t5 = sb("t5", [128, UT])
    hnew = sb("hnew", [128, UT])

    pe_fence = nc.alloc_psum_tensor("pe_fence", [128, 192], f32).ap()
    hT_p = nc.alloc_psum_tensor("hT_p", [2, 128], f32).ap()
    grp = nc.alloc_psum_tensor("grp", [128, UT], f32).ap()    # gi_r + gh_r
    gzp = nc.alloc_psum_tensor("gzp", [128, UT], f32).ap()    # gi_z + gh_z
    gin_p = nc.alloc_psum_tensor("gin_p", [128, UT], f32).ap()
    ghn_p = nc.alloc_psum_tensor("ghn_p", [128, UT], f32).ap()

    with contextlib.ExitStack() as _stack:
        sem = lambda n: _stack.enter_context(nc.semaphore(n))
        s_x = sem("s_x")
        s_h = sem("s_h")
        s_sm = sem("s_sm")
        s_wp = {key: sem("sw{}{}{}".format(*key)) for key in wp_d}
        s_ident = sem("s_ident")
        s_hT = sem("s_hT")
        s_gr = sem("s_gr")
        s_gn = sem("s_gn")
        s_gz = sem("s_gz")
        s_dve = sem("s_dve")
        s_act = sem("s_act")
        s_out = sem("s_out")
        block = _stack.enter_context(bare_block(nc))

        SYNC_PIECES = [
            ("r", "i", 0), ("r", "h", 1), ("n", "i", 0), ("n", "h", 1),
            ("z", "i", 0), ("z", "h", 1), ("z", "i", 2), ("z", "h", 3),
        ]
        SCALAR_PIECES = [
            ("r", "h", 0), ("r", "i", 1), ("n", "h", 0), ("n", "i", 1),
            ("z", "h", 0), ("z", "i", 1), ("z", "h", 2), ("z", "i", 3),
        ]
        # PE consumption order (matches arrival: piece j of each ring lands
        # ~together; r first, z last)
        CONSUME = [
            ("r", "i", 0), ("r", "h", 0), ("r", "i", 1), ("r", "h", 1),
            ("n", "i", 0), ("n", "h", 0), ("n", "i", 1), ("n", "h", 1),
            ("z", "i", 0), ("z", "h", 0), ("z", "i", 1), ("z", "h", 1),
            ("z", "i", 2), ("z", "h", 2), ("z", "i", 3), ("z", "h", 3),
        ]

        @block.sync
        def _(sync):
            sync.dma_start(out=xc[:, :], in_=xc_d.ap()[:, :]).then_inc(s_x, 16)
            sync.dma_start(out=sm[:, :], in_=smalls.ap()[:, :]).then_inc(s_sm, 16)
            for key in SYNC_PIECES:
                sync.dma_start(out=wp[key][:, :], in_=wp_d[key].ap()[:, :]).then_inc(
                    s_wp[key], 16
                )
            sync.dma_start(out=ident[:, :], in_=ident_d.ap()[:, :]).then_inc(
                s_ident, 16
            )

        @block.scalar
        def _(scalar):
            scalar.dma_start(out=hc[:, :], in_=hc_d.ap()[:, :]).then_inc(s_h, 16)
            for key in SCALAR_PIECES:
                scalar.dma_start(out=wp[key][:, :], in_=wp_d[key].ap()[:, :]).then_inc(
                    s_wp[key], 16
                )
            # r-gate sigmoid
            scalar.wait_ge(s_dve, 1)
            nc.scalar.activation(out=r_t[:, :], in_=rp[:, :], func=Act.Sigmoid).then_inc(
                s_act, 1
            )
            # n-gate tanh(v) = 2*sigmoid(2v) - 1, affine folded into DVE ops
            scalar.wait_ge(s_dve, 5)
            nc.scalar.activation(
                out=s_tile[:, :], in_=t4[:, :], func=Act.Sigmoid, scale=2.0
            ).then_inc(s_act, 1)
            # z-gate sigmoid
            scalar.wait_ge(s_dve, 8)
            nc.scalar.activation(out=z_t[:, :], in_=zp[:, :], func=Act.Sigmoid).then_inc(
                s_act, 1
            )
            # hnew arrives PE-transposed in hT_p [2, 128]: copy to SBUF and
            # ship as 2 fat descriptors instead of 128x8B (whose per-engine
            # HBM write receipts spread over ~3us)
            scalar.wait_ge(s_hT, 1)
            nc.scalar.activation(
                out=houtT[:, :], in_=hT_p[:, :], func=Act.Copy
            ).then_inc(s_act, 1)  # 4
            scalar.wait_ge(s_act, 4)
            scalar.dma_start(out=hout.ap()[:, :], in_=houtT[:, :]).then_inc(s_out, 16)
            scalar.wait_ge(s_out, 16)

        @block.tensor
        def _(tensor):
            def fence():
                wr0 = wp[("r", "i", 0)]
                return nc.tensor.matmul(
                    pe_fence[:, :],
                    lhsT=wr0[:, 0:128],
                    rhs=wr0[:, 0:192],
                    start=True,
                    stop=True,
                )

            def piece_pairs(key, acc, start, stop):
                wt = wp[key]
                vec = xc if key[1] == "i" else hc
                kt = PIECES[key] // HC           # k-tiles in this piece
                koff = key[2] * kt
                last = None
                for t in range(kt):
                    for j in range(UT):
                        last = nc.tensor.matmul(
                            acc[:, j : j + 1],
                            lhsT=wt[:, t * HC + j * 128 : t * HC + (j + 1) * 128],
                            rhs=vec[:, koff + t : koff + t + 1],
                            start=(start and t == 0 and j == 0),
                            stop=(stop and t == kt - 1),
                        )
                return last

            tensor.wait_ge(s_x, 16)
            tensor.wait_ge(s_h, 16)
            accs = {"r": (grp, grp), "n": (gin_p, ghn_p), "z": (gzp, gzp)}
            fences = {"r": s_gr, "n": s_gn, "z": s_gz}
            for gi, g in enumerate("rnz"):
                acc_i, acc_h = accs[g]
                fused = acc_i is acc_h
                kmax = 3 if g == "z" else 1
                for key in [k for k in CONSUME if k[0] == g]:
                    acc = acc_i if key[1] == "i" else acc_h
                    first_i = key[1] == "i" and key[2] == 0
                    first_h = key[1] == "h" and key[2] == 0
                    last_i = key[1] == "i" and key[2] == kmax
                    last_h = key[1] == "h" and key[2] == kmax
                    tensor.wait_ge(s_wp[key], 16)
                    piece_pairs(
                        key,
                        acc,
                        start=first_i if fused else (first_i or first_h),
                        stop=last_h if fused else (last_i or last_h),
                    )
                fence().then_inc(fences[g], 1)
            # transpose hnew [128, 2] -> hT_p [2, 128] for the fat-descriptor
            # output DMA
            tensor.wait_ge(s_ident, 16)
            tensor.wait_ge(s_dve, 10)
            nc.tensor.transpose(hT_p[:, :], hnew[:, :], ident[:, :])
            fence().then_inc(s_hT, 1)

        @block.vector
        def _(vector):
            vector.wait_ge(s_gr, 1)
            vector.wait_ge(s_sm, 16)
            nc.vector.tensor_tensor(
                out=rp[:, :], in0=grp[:, :], in1=brz_t[:, 0:UT], op=Alu.add
            ).then_inc(s_dve, 1)  # 1 -> ACT sigmoid(r)
            vector.wait_ge(s_gn, 1)
            nc.vector.tensor_tensor(
                out=hnb[:, :], in0=ghn_p[:, :], in1=bhn_t[:, :], op=Alu.add
            ).then_inc(s_dve, 1)  # 2
            nc.vector.tensor_tensor(
                out=t4[:, :], in0=gin_p[:, :], in1=bin_t[:, :], op=Alu.add
            ).then_inc(s_dve, 1)  # 3
            vector.wait_ge(s_act, 1)
            vector.wait_ge(s_dve, 3)
            nc.vector.tensor_tensor(
                out=t3[:, :], in0=r_t[:, :], in1=hnb[:, :], op=Alu.mult
            ).then_inc(s_dve, 1)  # 4
            vector.wait_ge(s_dve, 4)
            nc.vector.tensor_tensor(
                out=t4[:, :], in0=t4[:, :], in1=t3[:, :], op=Alu.add
            ).then_inc(s_dve, 1)  # 5 -> ACT sigmoid(2v)
            vector.wait_ge(s_act, 2)
            # u = 1 - 2s = -n
            nc.vector.tensor_scalar(
                out=n_t[:, :], in0=s_tile[:, :], scalar1=-2.0, scalar2=1.0,
                op0=Alu.mult, op1=Alu.add,
            ).then_inc(s_dve, 1)  # 6
            vector.wait_ge(s_dve, 6)
            nc.vector.tensor_tensor(
                out=t5[:, :], in0=hs_t[:, :], in1=n_t[:, :], op=Alu.add
            ).then_inc(s_dve, 1)  # 7  (t5 = hs - n)
            vector.wait_ge(s_gz, 1)
            nc.vector.tensor_tensor(
                out=zp[:, :], in0=gzp[:, :], in1=brz_t[:, UT : 2 * UT], op=Alu.add
            ).then_inc(s_dve, 1)  # 8 -> ACT sigmoid(z)
            vector.wait_ge(s_act, 3)
            vector.wait_ge(s_dve, 7)
            nc.vector.tensor_tensor(
                out=t5[:, :], in0=z_t[:, :], in1=t5[:, :], op=Alu.mult
            ).then_inc(s_dve, 1)  # 9  (t5 = z * (hs - n))
            vector.wait_ge(s_dve, 9)
            nc.vector.tensor_tensor(
                out=hnew[:, :], in0=t5[:, :], in1=n_t[:, :], op=Alu.subtract
            ).then_inc(s_dve, 1)  # 10  (hnew = n + z*(hs - n))

    nc.compile()

    # Post-compile surgery:
    # 1. Strip the entry all-engine barrier (per-engine Drain + barrier_*
    #    EventSemaphores in the entry block). The only ordering it provides
    #    is Pool's preamble memsets vs other engines' const-AP reads; the
    #    first such read (ACT sigmoid bias) is ~15us after the ~1us memsets.
    blk0 = nc.main_func.blocks[0]
    kill = [
        i
        for i in blk0.instructions
        if isinstance(i, mybir.InstDrain)
        or (isinstance(i, mybir.InstEventSemaphore) and "barrier_" in str(i))
    ]
    assert len(kill) >= 10, f"expected entry barrier instrs, got {len(kill)}"
    for i in kill:
        blk0.instructions.remove(i)
    # 2. Move the entry LoadActFuncSet after the scalar-ring DMA issues so
    #    its 1.3us table load does not delay the ring start; the set that
    #    the sigmoids use loads right before them anyway.
    for b in nc.main_func.blocks:
        loads = [i for i in b.instructions if isinstance(i, mybir.InstLoadActFuncSet)]
        if len(loads) >= 2:
            first = loads[0]
            b.instructions.remove(first)
            acts = [
                idx
                for idx, i in enumerate(b.instructions)
                if isinstance(i, (mybir.InstActivation, mybir.InstLoadActFuncSet))
            ]
            b.instructions.insert(acts[0], first)
    return nc


def get_nc():
    if "nc" not in _CACHE:
        _CACHE["nc"] = _build()
    return _CACHE["nc"]


def make_in_maps(inputs):
    """Host-side sharding: full-input dict -> 8 per-core input maps."""
    emb = np.asarray(inputs["emb"], dtype=np.float32)
    w_ih = np.asarray(inputs["w_ih"], dtype=np.float32)
    w_hh = np.asarray(inputs["w_hh"], dtype=np.float32)
    b_ih = np.asarray(inputs["b_ih"], dtype=np.float32)
    b_hh = np.asarray(inputs["b_hh"], dtype=np.float32)
    idx = int(np.asarray(inputs["input"]).reshape(-1)[0])
    x = np.ascontiguousarray(emb[idx])
    h = np.asarray(inputs["hidden"], dtype=np.float32).reshape(H)

    xc_host = np.ascontiguousarray(x.reshape(KC, 128).T.astype(np.float16))
    hc_host = np.ascontiguousarray(h.reshape(KC, 128).T.astype(np.float16))
    bsum = b_ih + b_hh

    in_maps = []
    for c in range(NCORES):
        # per-core row slices, PyTorch gate order r, z, n
        sl = [slice(g * H + c * HC, g * H + c * HC + HC) for g in range(3)]
        r_sl, z_sl, n_sl = sl[0], sl[1], sl[2]

        # k-major gate-matrix image [128, 16*256]:
        # img[p, t*256 + o] = G^T[t*128 + p, o]
        def img(m, g_sl):
            rows = m[g_sl].T.astype(np.float16)          # [2048, 256]
            return rows.reshape(KC, 128, HC).transpose(1, 0, 2).reshape(128, GMW)

        slabs = {}
        for g, g_slc in (("r", r_sl), ("n", n_sl), ("z", z_sl)):
            np_pieces = 4 if g == "z" else 2
            w = GMW // np_pieces
            for m, mat in (("i", w_ih), ("h", w_hh)):
                full = img(mat, g_slc)
                for k in range(np_pieces):
                    slabs[f"w{g}{m}{k}"] = np.ascontiguousarray(
                        full[:, k * w : (k + 1) * w]
                    )
        brz_c = np.concatenate([bsum[r_sl], bsum[z_sl]]).reshape(2 * UT, 128).T
        bin_c = b_ih[n_sl].reshape(UT, 128).T
        bhn_c = b_hh[n_sl].reshape(UT, 128).T
        hs_c = h[c * HC : (c + 1) * HC].reshape(UT, 128).T
        smalls_c = np.ascontiguousarray(
            np.concatenate([brz_c, bin_c, bhn_c, hs_c], axis=1), dtype=np.float32
        )
        in_maps.append(
            {
                **slabs,
                "xc": xc_host,
                "hc": hc_host,
                "smalls": smalls_c,
                "ident": np.eye(128, dtype=np.float32),
            }
        )
    return in_maps


def run_on_hw(in_maps, trace=False):
    from concourse.bass_utils import run_bass_kernel_spmd

    kwargs = {}
    if trace:
        kwargs.update(trace=True, trace_cores=list(range(NCORES)))
    return run_bass_kernel_spmd(get_nc(), in_maps, core_ids=list(range(NCORES)), **kwargs)


def assemble(results):
    h_new = np.concatenate(
        [np.ascontiguousarray(results[c]["hout"]).reshape(HC) for c in range(NCORES)]
    )
    out = h_new.reshape(1, 1, H).astype(np.float32)
    return out, out.copy()


def kernel(**inputs):
    in_maps = make_in_maps(inputs)
    res = run_on_hw(in_maps)
    return assemble(res.results)


# revision 35
# speedup vs baseline: 1.3158x; 1.0093x over previous
"""GRU cell (EncoderRNN single step) on 8 Trainium2 NeuronCores.

Full inputs -> full output. Sharding: each core owns a 256-wide slice of the
hidden dimension across all three gates (rows of w_ih/w_hh); no collectives.
The host gathers the embedding row (only that row of the table is needed) and
concatenates the 8 per-core h_new slices.

All matrix-vector work runs on the PE array via host-transposed k-major fp16
weights (measured LDWEIGHTS/MATMUL pair for [128,128]x[128,1]: ~27ns warm, so
the 384 pairs/core hide under the weight stream). DVE only does ten [128,2]
gate ops; ACT does the three sigmoids (tanh(v) = 2*sigmoid(2v)-1 keeps a
single act-table set).

Streaming: 6.29MB/core of fp16 weights over THREE DMA rows - the 16 SDMA
engines round-robin between rows at packet granularity, and a single row
leaves them ~50% idle on descriptor-packet fetch, so more rows = more
overlap. r rides first on the two HWDGE rings, z last on them, n on the
Pool SWDGE row (which starts earlier and lands mid-stream), so each gate's
sigmoid chain overlaps the next gate's stream.

Hard-won correctness rules (cold-run races otherwise, masked on reruns by
stale-but-identical SBUF/PSUM contents):
- ONE semaphore per DMA. A shared sem with >=16*(c+1) thresholds is unsound:
  the 16 SDMA engines' receipt increments all land in one counter, so a
  straggler engine can still be writing chunk c while faster engines push
  the sum past the threshold.
- The gate-complete sem must not fire before the last matmul's ~128-cycle
  PSUM drain lands: a dummy 192-column matmul (fence) carries the inc.
- start=True clears the has_written state for the whole 2KB PSUM bank
  region: only the FIRST matmul of each accumulator bank sets it.
"""

import sys

if "/opt/trn_rl_repo" not in sys.path:
    sys.path.insert(0, "/opt/trn_rl_repo")

import numpy as np
import ml_dtypes

H = 2048
NCORES = 8
HC = H // NCORES          # 256 hidden elems per core
UT = HC // 128            # 2 columns for the per-core [128, 2] gate slices
KC = H // 128             # 16 k-chunks
GMW = KC * HC             # 4096 cols per gate-matrix image ([128, 4096] f16)

_CACHE = {}


def _build():
    import contextlib
    from concourse import bacc, bass, mybir

    class _BareBlock(bass.BassBlock):
        # Skip the exit drains + all-engine EVSEM barrier: every cross-engine
        # dependency is semaphore-guarded and the issuing engine of the hout
        # DMA waits for its receipt, so nothing needs a terminal rendezvous.
        def __exit__(self, exc_type, exc_val, exc_tb):
            if exc_type is None:
                for engine, last_body in self.last_body.items():
                    with self.bass.body(
                        last_body, parent=self.bass.cur_bb, allow_existing_parent=True
                    ):
                        engine.br(self.end_bb)
                self.bass.switch_bb(self.end_bb)

    @contextlib.contextmanager
    def bare_block(nc):
        assert nc.cur_block is None
        with _BareBlock(nc, f"block_{nc.next_id()}") as blk:
            nc.cur_block = blk
            yield blk
        nc.cur_block = None

    f32 = mybir.dt.float32
    f16 = mybir.dt.float16
    Alu = mybir.AluOpType
    Act = mybir.ActivationFunctionType

    nc = bacc.Bacc(
        "TRN2",
        target_bir_lowering=False,
        debug=False,
        num_devices=NCORES,
        detect_race_conditions=False,
    )

    # one CONTIGUOUS DRAM tensor per half-slab (512KB, 4KB rows): wider
    # concatenated images make descriptors skip bytes and halve DRAM page
    # locality. 12 pieces, alternating rings, bound the damage from
    # run-to-run ring-rate skew to half a slab.
    # piece (g, m, k): gate g in (r,n,z), matrix m in (ih,hh), k-half
    # r/n in 512KB halves; z in 256KB quarters so the last-arriving piece
    # only gates ~8 PE pairs of tail work
    def _pieces():
        for g in "rn":
            for m in "ih":
                for k in (0, 1):
                    yield (g, m, k), GMW // 2
        for m in "ih":
            for k in (0, 1, 2, 3):
                yield ("z", m, k), GMW // 4

    PIECES = dict(_pieces())
    wp_d = {
        key: nc.dram_tensor("w{}{}{}".format(*key), [128, w], f16,
                            kind="ExternalInput")
        for key, w in PIECES.items()
    }
    ident_d = nc.dram_tensor("ident", [128, 128], f32, kind="ExternalInput")
    xc_d = nc.dram_tensor("xc", [128, KC], f16, kind="ExternalInput")
    hc_d = nc.dram_tensor("hc", [128, KC], f16, kind="ExternalInput")
    # cols: brz[0:2*UT], bin[2*UT:3*UT], bhn[3*UT:4*UT], hs[4*UT:5*UT]
    smalls = nc.dram_tensor("smalls", [128, 5 * UT], f32, kind="ExternalInput")
    hout = nc.dram_tensor("hout", [UT, 128], f32, kind="ExternalOutput")

    sb = lambda name, shape, dt=f32: nc.alloc_sbuf_tensor(name, list(shape), dt).ap()
    wp = {
        key: sb("s_w{}{}{}".format(*key), [128, PIECES[key]], f16)
        for key in wp_d
    }
    ident = sb("ident_s", [128, 128], f32)
    houtT = sb("houtT", [2, 128])
    xc = sb("xc_s", [128, KC], f16)
    hc = sb("hc_s", [128, KC], f16)
    sm = sb("sm", [128, 5 * UT])
    brz_t = sm[:, 0 : 2 * UT]
    bin_t = sm[:, 2 * UT : 3 * UT]
    bhn_t = sm[:, 3 * UT : 4 * UT]
    hs_t = sm[:, 4 * UT : 5 * UT]
    rp = sb("rp", [128, UT])
    zp = sb("zp", [128, UT])
    r_t = sb("r_t", [128, UT])
    z_t = sb("z_t", [128, UT])
    hnb = sb("hnb", [128, UT])
    t3 = sb("t3", [128, UT])
    t4 = sb("t4", [128, UT])
    s_tile = sb("s_tile", [128, UT])   # sigmoid(2v) for the n gate
    n_t = sb("n_t", [128, UT])         # u = 1 - 2s = -n
    t5 = sb("t5", [128, UT])
    hnew = sb("hnew", [128, UT])

    pe_fence = nc.alloc_psum_tensor("pe_fence", [128, 192], f32).ap()
    hT_p = nc.alloc_psum_tensor("hT_p", [2, 128], f32).ap()
    grp = nc.alloc_psum_tensor("grp", [128, UT], f32).ap()    # gi_r + gh_r
    gzp = nc.alloc_psum_tensor("gzp", [128, UT], f32).ap()    # gi_z + gh_z
    gin_p = nc.alloc_psum_tensor("gin_p", [128, UT], f32).ap()
    ghn_p = nc.alloc_psum_tensor("ghn_p", [128, UT], f32).ap()

    with contextlib.ExitStack() as _stack:
        sem = lambda n: _stack.enter_context(nc.semaphore(n))
        s_x = sem("s_x")
        s_h = sem("s_h")
        s_sm = sem("s_sm")
        s_wp = {key: sem("sw{}{}{}".format(*key)) for key in wp_d}
        s_ident = sem("s_ident")
        s_hT = sem("s_hT")
        s_gr = sem("s_gr")
        s_gn = sem("s_gn")
        s_gz = sem("s_gz")
        s_dve = sem("s_dve")
        s_act = sem("s_act")
        s_out = sem("s_out")
        block = _stack.enter_context(bare_block(nc))

        SYNC_PIECES = [
            ("r", "i", 0), ("r", "h", 1), ("n", "i", 0), ("n", "h", 1),
            ("z", "i", 0), ("z", "h", 1), ("z", "i", 2), ("z", "h", 3),
        ]
        SCALAR_PIECES = [
            ("r", "h", 0), ("r", "i", 1), ("n", "h", 0), ("n", "i", 1),
            ("z", "h", 0), ("z", "i", 1), ("z", "h", 2), ("z", "i", 3),
        ]
        # PE consumption order (matches arrival: piece j of each ring lands
        # ~together; r first, z last)
        CONSUME = [
            ("r", "i", 0), ("r", "h", 0), ("r", "i", 1), ("r", "h", 1),
            ("n", "i", 0), ("n", "h", 0), ("n", "i", 1), ("n", "h", 1),
            ("z", "i", 0), ("z", "h", 0), ("z", "i", 1), ("z", "h", 1),
            ("z", "i", 2), ("z", "h", 2), ("z", "i", 3), ("z", "h", 3),
        ]

        @block.sync
        def _(sync):
            sync.dma_start(out=xc[:, :], in_=xc_d.ap()[:, :]).then_inc(s_x, 16)
            sync.dma_start(out=sm[:, :], in_=smalls.ap()[:, :]).then_inc(s_sm, 16)
            for key in SYNC_PIECES:
                sync.dma_start(out=wp[key][:, :], in_=wp_d[key].ap()[:, :]).then_inc(
                    s_wp[key], 16
                )
            sync.dma_start(out=ident[:, :], in_=ident_d.ap()[:, :]).then_inc(
                s_ident, 16
            )
            sync.wait_ge(s_act, 4)
            sync.dma_start(out=hout.ap()[:, :], in_=houtT[:, :]).then_inc(s_out, 16)
            sync.wait_ge(s_out, 16)

        @block.scalar
        def _(scalar):
            scalar.dma_start(out=hc[:, :], in_=hc_d.ap()[:, :]).then_inc(s_h, 16)
            for key in SCALAR_PIECES:
                scalar.dma_start(out=wp[key][:, :], in_=wp_d[key].ap()[:, :]).then_inc(
                    s_wp[key], 16
                )
            # r-gate sigmoid
            scalar.wait_ge(s_dve, 1)
            nc.scalar.activation(out=r_t[:, :], in_=rp[:, :], func=Act.Sigmoid).then_inc(
                s_act, 1
            )
            # n-gate tanh(v) = 2*sigmoid(2v) - 1, affine folded into DVE ops
            scalar.wait_ge(s_dve, 5)
            nc.scalar.activation(
                out=s_tile[:, :], in_=t4[:, :], func=Act.Sigmoid, scale=2.0
            ).then_inc(s_act, 1)
            # z-gate sigmoid
            scalar.wait_ge(s_dve, 8)
            nc.scalar.activation(out=z_t[:, :], in_=zp[:, :], func=Act.Sigmoid).then_inc(
                s_act, 1
            )
            # hnew arrives PE-transposed in hT_p [2, 128]: copy to SBUF and
            # ship as 2 fat descriptors instead of 128x8B (whose per-engine
            # HBM write receipts spread over ~3us)
            scalar.wait_ge(s_hT, 1)
            nc.scalar.activation(
                out=houtT[:, :], in_=hT_p[:, :], func=Act.Copy
            ).then_inc(s_act, 1)  # 4


        @block.tensor
        def _(tensor):
            def fence():
                wr0 = wp[("r", "i", 0)]
                return nc.tensor.matmul(
                    pe_fence[:, :],
                    lhsT=wr0[:, 0:128],
                    rhs=wr0[:, 0:192],
                    start=True,
                    stop=True,
                )

            def piece_pairs(key, acc, start, stop):
                wt = wp[key]
                vec = xc if key[1] == "i" else hc
                kt = PIECES[key] // HC           # k-tiles in this piece
                koff = key[2] * kt
                last = None
                for t in range(kt):
                    for j in range(UT):
                        last = nc.tensor.matmul(
                            acc[:, j : j + 1],
                            lhsT=wt[:, t * HC + j * 128 : t * HC + (j + 1) * 128],
                            rhs=vec[:, koff + t : koff + t + 1],
                            start=(start and t == 0 and j == 0),
                            stop=(stop and t == kt - 1),
                        )
                return last

            tensor.wait_ge(s_x, 16)
            tensor.wait_ge(s_h, 16)
            accs = {"r": (grp, grp), "n": (gin_p, ghn_p), "z": (gzp, gzp)}
            fences = {"r": s_gr, "n": s_gn, "z": s_gz}
            for gi, g in enumerate("rnz"):
                acc_i, acc_h = accs[g]
                fused = acc_i is acc_h
                kmax = 3 if g == "z" else 1
                for key in [k for k in CONSUME if k[0] == g]:
                    acc = acc_i if key[1] == "i" else acc_h
                    first_i = key[1] == "i" and key[2] == 0
                    first_h = key[1] == "h" and key[2] == 0
                    last_i = key[1] == "i" and key[2] == kmax
                    last_h = key[1] == "h" and key[2] == kmax
                    tensor.wait_ge(s_wp[key], 16)
                    piece_pairs(
                        key,
                        acc,
                        start=first_i if fused else (first_i or first_h),
                        stop=last_h if fused else (last_i or last_h),
                    )
                fence().then_inc(fences[g], 1)
            # transpose hnew [128, 2] -> hT_p [2, 128] for the fat-descriptor
            # output DMA
            tensor.wait_ge(s_ident, 16)
            tensor.wait_ge(s_dve, 10)
            nc.tensor.transpose(hT_p[:, :], hnew[:, :], ident[:, :])
            fence().then_inc(s_hT, 1)

        @block.vector
        def _(vector):
            vector.wait_ge(s_gr, 1)
            vector.wait_ge(s_sm, 16)
            nc.vector.tensor_tensor(
                out=rp[:, :], in0=grp[:, :], in1=brz_t[:, 0:UT], op=Alu.add
            ).then_inc(s_dve, 1)  # 1 -> ACT sigmoid(r)
            vector.wait_ge(s_gn, 1)
            nc.vector.tensor_tensor(
                out=hnb[:, :], in0=ghn_p[:, :], in1=bhn_t[:, :], op=Alu.add
            ).then_inc(s_dve, 1)  # 2
            nc.vector.tensor_tensor(
                out=t4[:, :], in0=gin_p[:, :], in1=bin_t[:, :], op=Alu.add
            ).then_inc(s_dve, 1)  # 3
            vector.wait_ge(s_act, 1)
            vector.wait_ge(s_dve, 3)
            nc.vector.tensor_tensor(
                out=t3[:, :], in0=r_t[:, :], in1=hnb[:, :], op=Alu.mult
            ).then_inc(s_dve, 1)  # 4
            vector.wait_ge(s_dve, 4)
            nc.vector.tensor_tensor(
                out=t4[:, :], in0=t4[:, :], in1=t3[:, :], op=Alu.add
            ).then_inc(s_dve, 1)  # 5 -> ACT sigmoid(2v)
            vector.wait_ge(s_act, 2)
            # u = 1 - 2s = -n
            nc.vector.tensor_scalar(
                out=n_t[:, :], in0=s_tile[:, :], scalar1=-2.0, scalar2=1.0,
                op0=Alu.mult, op1=Alu.add,
            ).then_inc(s_dve, 1)  # 6
            vector.wait_ge(s_dve, 6)
            nc.vector.tensor_tensor(
                out=t5[:, :], in0=hs_t[:, :], in1=n_t[:, :], op=Alu.add
            ).then_inc(s_dve, 1)  # 7  (t5 = hs - n)
            vector.wait_ge(s_gz, 1)
            nc.vector.tensor_tensor(
                out=zp[:, :], in0=gzp[:, :], in1=brz_t[:, UT : 2 * UT], op=Alu.add
            ).then_inc(s_dve, 1)  # 8 -> ACT sigmoid(z)
            vector.wait_ge(s_act, 3)
            vector.wait_ge(s_dve, 7)
            nc.vector.tensor_tensor(
                out=t5[:, :], in0=z_t[:, :], in1=t5[:, :], op=Alu.mult
            ).then_inc(s_dve, 1)  # 9  (t5 = z * (hs - n))
            vector.wait_ge(s_dve, 9)
            nc.vector.tensor_tensor(
                out=hnew[:, :], in0=t5[:, :], in1=n_t[:, :], op=Alu.subtract
            ).then_inc(s_dve, 1)  # 10  (hnew = n + z*(hs - n))

    nc.compile()

    # Post-compile surgery:
    # 1. Strip the entry all-engine barrier (per-engine Drain + barrier_*
    #    EventSemaphores in the entry block). The only ordering it provides
    #    is Pool's preamble memsets vs other engines' const-AP reads; the
    #    first such read (ACT sigmoid bias) is ~15us after the ~1us memsets.
    blk0 = nc.main_func.blocks[0]
    kill = [
        i
        for i in blk0.instructions
        if isinstance(i, mybir.InstDrain)
        or (isinstance(i, mybir.InstEventSemaphore) and "barrier_" in str(i))
    ]
    assert len(kill) >= 10, f"expected entry barrier instrs, got {len(kill)}"
    for i in kill:
        blk0.instructions.remove(i)
    # 2. Move the entry LoadActFuncSet after the scalar-ring DMA issues so
    #    its 1.3us table load does not delay the ring start; the set that
    #    the sigmoids use loads right before them anyway.
    for b in nc.main_func.blocks:
        loads = [i for i in b.instructions if isinstance(i, mybir.InstLoadActFuncSet)]
        if len(loads) >= 2:
            first = loads[0]
            b.instructions.remove(first)
            acts = [
                idx
                for idx, i in enumerate(b.instructions)
                if isinstance(i, (mybir.InstActivation, mybir.InstLoadActFuncSet))
            ]
            b.instructions.insert(acts[0], first)
    return nc


def get_nc():
    if "nc" not in _CACHE:
        _CACHE["nc"] = _build()
    return _CACHE["nc"]


def make_in_maps(inputs):
    """Host-side sharding: full-input dict -> 8 per-core input maps."""
    emb = np.asarray(inputs["emb"], dtype=np.float32)
    w_ih = np.asarray(inputs["w_ih"], dtype=np.float32)
    w_hh = np.asarray(inputs["w_hh"], dtype=np.float32)
    b_ih = np.asarray(inputs["b_ih"], dtype=np.float32)
    b_hh = np.asarray(inputs["b_hh"], dtype=np.float32)
    idx = int(np.asarray(inputs["input"]).reshape(-1)[0])
    x = np.ascontiguousarray(emb[idx])
    h = np.asarray(inputs["hidden"], dtype=np.float32).reshape(H)

    xc_host = np.ascontiguousarray(x.reshape(KC, 128).T.astype(np.float16))
    hc_host = np.ascontiguousarray(h.reshape(KC, 128).T.astype(np.float16))
    bsum = b_ih + b_hh

    in_maps = []
    for c in range(NCORES):
        # per-core row slices, PyTorch gate order r, z, n
        sl = [slice(g * H + c * HC, g * H + c * HC + HC) for g in range(3)]
        r_sl, z_sl, n_sl = sl[0], sl[1], sl[2]

        # k-major gate-matrix image [128, 16*256]:
        # img[p, t*256 + o] = G^T[t*128 + p, o]
        def img(m, g_sl):
            rows = m[g_sl].T.astype(np.float16)          # [2048, 256]
            return rows.reshape(KC, 128, HC).transpose(1, 0, 2).reshape(128, GMW)

        slabs = {}
        for g, g_slc in (("r", r_sl), ("n", n_sl), ("z", z_sl)):
            np_pieces = 4 if g == "z" else 2
            w = GMW // np_pieces
            for m, mat in (("i", w_ih), ("h", w_hh)):
                full = img(mat, g_slc)
                for k in range(np_pieces):
                    slabs[f"w{g}{m}{k}"] = np.ascontiguousarray(
                        full[:, k * w : (k + 1) * w]
                    )
        brz_c = np.concatenate([bsum[r_sl], bsum[z_sl]]).reshape(2 * UT, 128).T
        bin_c = b_ih[n_sl].reshape(UT, 128).T
        bhn_c = b_hh[n_sl].reshape(UT, 128).T
        hs_c = h[c * HC : (c + 1) * HC].reshape(UT, 128).T
        smalls_c = np.ascontiguousarray(
            np.concatenate([brz_c, bin_c, bhn_c, hs_c], axis=1), dtype=np.float32
        )
        in_maps.append(
            {
                **slabs,
                "xc": xc_host,
                "hc": hc_host,
                "smalls": smalls_c,
                "ident": np.eye(128, dtype=np.float32),
            }
        )
    return in_maps


def run_on_hw(in_maps, trace=False):
    from concourse.bass_utils import run_bass_kernel_spmd

    kwargs = {}
    if trace:
        kwargs.update(trace=True, trace_cores=list(range(NCORES)))
    return run_bass_kernel_spmd(get_nc(), in_maps, core_ids=list(range(NCORES)), **kwargs)


def assemble(results):
    h_new = np.concatenate(
        [np.ascontiguousarray(results[c]["hout"]).reshape(HC) for c in range(NCORES)]
    )
    out = h_new.reshape(1, 1, H).astype(np.float32)
    return out, out.copy()


def kernel(**inputs):
    in_maps = make_in_maps(inputs)
    res = run_on_hw(in_maps)
    return assemble(res.results)
